# revision 27
# baseline (speedup 1.0000x reference)
"""Trainium2 Bass kernel for the nn_Adaptor problem.

Computation (per batch image):
  avgpool4x4 -> GN(32 groups)+SiLU -> conv3x3 320->8 -> attention(4 heads) ->
  per-pixel LN + MLP -> GN(8)+SiLU -> upsample x4 nearest -> conv3x3 8->320

Distribution: pure data parallel over batch. 16 images / 8 cores = 2 per core.
Params are baked into the NEFF as inline consts (recomputed from the numpy
arrays passed to kernel() at trace time).

Implementation notes:
  - pooling keeps raw 4x4 sums (16x scale); GN1 uses eps_eff = 256*eps so the
    normalized output is exact.
  - GN1 group stats via per-channel bn_stats + grouping-matrix matmuls on PE.
  - All norm+SiLU applications fused into single scalar-engine activations.
  - conv1 as 9 shifted-window matmuls over a zero-padded 18x18 tile; both
    local batch images stacked along the matmul free dim (N=512).
  - attention: transposed scores E^T = exp(k^T q) without max subtraction
    (|scores| < 0.5 for this operator family); two heads per matmul via
    zero-masked q blocks; softmax denominators via ones-matmul column sums;
    head gather folded into zero-masked projection matmuls.
  - GN2 applied pre-upsample (nearest-upsample preserves per-channel stats).
  - conv2-after-upsample collapses to 9 phase groups with collapsed weights on
    the 16x16 grid (K=72 over a 9-slot shifted-window stack); phase outputs are
    interleaved on-chip by strided copies with step-0 column duplication, then
    written out with fully contiguous DMAs split across both HWDGE rings.
  - the middle is one batch-stacked dependency chain; engine streams execute
    in order, so fewer/wider ops beat two interleaved per-batch chains.
"""

import ml_dtypes
import numpy as np

import concourse.bass as bass
import concourse.bacc as bacc
import concourse.tile as tile
from concourse import mybir
from concourse.bass_utils import run_bass_kernel_spmd

F32 = mybir.dt.float32
BF16 = mybir.dt.bfloat16
NPBF = ml_dtypes.bfloat16
AF = mybir.ActivationFunctionType
ALU = mybir.AluOpType
AX = mybir.AxisListType

CH, C, D, HEADS = 320, 4, 8, 4
EPS = 1e-5
B = 2
N = 256
NCORES = 8

TAPSETS = {
    0: [(0, (0,)), (1, (1, 2))],
    1: [(1, (0, 1, 2))],
    3: [(1, (0, 1)), (2, (2,))],
}
GROUPS = [(pi, pj) for pi in (0, 1, 3) for pj in (0, 1, 3)]
ROWSETS = {0: (0, 1), 1: (1, 2), 3: (3, 1)}   # (start row, duplication count)

KERNEL_TRACE = False


def _conv2_tables(w_conv2):
    """W9 [72, 9, 3, 128]: collapsed per-phase-group weights over the 9-slot
    shifted-window stack; chunk 2 duplicated into cols 64:128 (two-batch
    chunk-2 matmul keeps batch 1 at psum partitions 64:128)."""
    W9 = np.zeros((72, 9, 3, 128), np.float32)
    for g, (pi, pj) in enumerate(GROUPS):
        for (dy, kys) in TAPSETS[pi]:
            for (dx, kxs) in TAPSETS[pj]:
                s = 3 * dy + dx
                wsum = np.zeros((CH, D), np.float32)
                for ky in kys:
                    for kx in kxs:
                        wsum += w_conv2[:, :, ky, kx]
                for c in range(3):
                    oc0 = 128 * c
                    ocn = min(128, CH - oc0)
                    blk = wsum[oc0:oc0 + ocn].T
                    W9[8 * s:8 * s + 8, g, c, :ocn] += blk
                    if c == 2:
                        W9[8 * s:8 * s + 8, g, c, 64:64 + ocn] += blk
    return W9


def build(params):
    P = params
    nc = bacc.Bacc("TRN2")

    x = nc.dram_tensor("x", [B, CH, 64, 64], F32, kind="ExternalInput")
    out = nc.dram_tensor("out", [B, CH, 64, 64], F32, kind="ExternalOutput")

    # ---------------- host-side constant prep ----------------
    s = float(1 / np.sqrt(D // HEADS))
    wq = P["w_qkv"].copy()
    bq = P["b_qkv"].copy()
    wq[D:2 * D] *= s
    bq[D:2 * D] *= s

    W1 = np.zeros((128, 3, 9, 8), np.float32)
    for c in range(3):
        c0 = 128 * c
        cn = min(128, CH - c0)
        for ky in range(3):
            for kx in range(3):
                W1[:cn, c, 3 * ky + kx, :] = P["w_conv1"][:, c0:c0 + cn, ky, kx].T

    W9 = _conv2_tables(P["w_conv2"])

    Gsum = np.zeros((128, 3, 32), np.float32)
    GT = np.zeros((32, 3, 128), np.float32)
    for c in range(CH):
        k, p = divmod(c, 128)
        Gsum[p, k, c // 10] = 0.1
        GT[c // 10, k, p] = 1.0

    def chunks(v, dup2=False):
        a = np.zeros((128, 3), np.float32)
        for c in range(3):
            c0 = 128 * c
            cn = min(128, CH - c0)
            a[:cn, c] = v[c0:c0 + cn]
            if dup2 and c == 2:
                a[64:64 + cn, c] = v[c0:c0 + cn]
        return a

    vch = np.stack([chunks(P["g1"]), chunks(P["b1"]),
                    chunks(P["b_conv2"], dup2=True)], axis=2)  # [128, 3, 3]

    cols8, pk8 = {}, []

    def pack8(name, arr):
        arr = np.asarray(arr, np.float32).reshape(8, -1)
        cols8[name] = (sum(a.shape[1] for a in pk8), arr.shape[1])
        pk8.append(arr)

    pack8("ones8", np.full((8, 1), 0.125, np.float32))
    for nm, val in [("bq8", bq[0:D]), ("bk8", bq[D:2 * D]), ("bv", bq[2 * D:]),
                    ("b_conv1", P["b_conv1"]), ("b_proj", P["b_proj"]),
                    ("ln_g", P["ln_g"]), ("ln_b", P["ln_b"]),
                    ("b_fc2", P["b_fc2"]), ("ga", P["ga"]), ("ba", P["ba"]),
                    ("g2", P["g2"]), ("b2", P["b2"])]:
        pack8(nm, val.reshape(8, 1))
    PK8 = np.concatenate(pk8, axis=1)

    # wide-middle consts: batch b lives at partition base 32*b
    def widen(v8, n=40):
        a = np.zeros((n, 1), np.float32)
        a[0:8, 0] = v8
        a[32:40, 0] = v8
        return a

    def blockdiag(w, n_in=40, n_out=40):
        a = np.zeros((n_in, n_out), np.float32)
        r, c = w.shape
        a[0:r, 0:c] = w
        a[32:32 + r, 32:32 + c] = w
        return a

    cols40, pk40 = {}, []

    def pack40(name, arr):
        arr = np.asarray(arr, np.float32).reshape(40, -1)
        cols40[name] = (sum(a.shape[1] for a in pk40), arr.shape[1])
        pk40.append(arr)

    for nm, val in [("bq8", bq[0:D]), ("bk8", bq[D:2 * D]), ("bv", bq[2 * D:]),
                    ("b_conv1", P["b_conv1"]), ("b_proj", P["b_proj"]),
                    ("ln_g", P["ln_g"]), ("ln_b", P["ln_b"]),
                    ("b_fc2", P["b_fc2"]), ("ga", P["ga"]), ("ba", P["ba"]),
                    ("g2", P["g2"]), ("b2", P["b2"])]:
        pack40(nm, widen(val.reshape(8)))
    PK40 = np.concatenate(pk40, axis=1)

    cols40b, pk40b = {}, []

    def pack40b(name, arr):
        arr = np.asarray(arr, np.float32)
        arr = arr.reshape(40, -1)
        cols40b[name] = (sum(a.shape[1] for a in pk40b), arr.shape[1])
        pk40b.append(arr)

    pack40b("wqBD", blockdiag(wq[0:D].T))
    pack40b("wkBD", blockdiag(wq[D:2 * D].T))
    pack40b("wvBD", blockdiag(P["w_qkv"][2 * D:3 * D].T))
    identW = np.zeros((40, 8), np.float32)
    identW[0:8] = np.eye(8)
    identW[32:40] = np.eye(8)
    pack40b("identW", identW)
    qmaskW = np.zeros((40, 4), np.float32)
    for c in range(8):
        qmaskW[c, c // 2] = 1.0
        qmaskW[32 + c, c // 2] = 1.0
    pack40b("qmaskW", qmaskW)
    wpHW = np.zeros((40, 4, 8), np.float32)
    for c in range(8):
        wpHW[c, c // 2, :] = P["w_proj"][:, c]
        wpHW[32 + c, c // 2, :] = P["w_proj"][:, c]
    pack40b("wpHW", wpHW.reshape(40, 32))
    pack40b("wf1BD", blockdiag(P["w_fc1"].T, 40, 48))
    w8BD = np.zeros((40, 2), np.float32)
    w8BD[0:8, 0] = 0.125
    w8BD[32:40, 1] = 0.125
    pack40b("w8BD", w8BD)
    PK40B = np.concatenate(pk40b, axis=1).astype(NPBF)

    WF2BD = blockdiag(P["w_fc2"].T, 48, 40).astype(NPBF)   # [48, 40]
    BF1W = np.zeros((48, 1), np.float32)
    BF1W[0:16, 0] = P["b_fc1"]
    BF1W[32:48, 0] = P["b_fc1"]
    SEL2W = np.zeros((2, 40), np.float32)
    SEL2W[0, 0:8] = 1.0
    SEL2W[1, 32:40] = 1.0
    SEL2W = SEL2W.astype(NPBF)

    cols8b, pk8b = {}, []

    def pack8b(name, arr):
        arr = np.asarray(arr, np.float32).reshape(8, -1)
        cols8b[name] = (sum(a.shape[1] for a in pk8b), arr.shape[1])
        pk8b.append(arr)

    pack8b("wqT", wq[0:D].T)
    pack8b("wkT", wq[D:2 * D].T)
    pack8b("wvT", P["w_qkv"][2 * D:3 * D].T)
    pack8b("ident8", np.eye(8, dtype=np.float32))
    qmask = np.zeros((8, 4), np.float32)
    for c in range(8):
        qmask[c, c // 2] = 1.0
    pack8b("qmask", qmask)
    wpH = np.zeros((8, 4, 8), np.float32)
    for c in range(8):
        wpH[c, c // 2, :] = P["w_proj"][:, c]
    pack8b("wpH", wpH.reshape(8, 32))
    pack8b("wf1T", P["w_fc1"].T)
    PK8B = np.concatenate(pk8b, axis=1).astype(NPBF)

    PK16 = np.concatenate([P["w_fc2"].T, P["b_fc1"].reshape(16, 1)], axis=1)
    WF2B = P["w_fc2"].T.astype(NPBF)

    h_w1 = nc.inline_tensor(W1.astype(NPBF), "cW1")
    h_w9 = nc.inline_tensor(W9.astype(NPBF), "cW9")
    h_gsum = nc.inline_tensor(Gsum, "cGsum")
    h_gt = nc.inline_tensor(GT, "cGT")
    h_vch = nc.inline_tensor(vch, "cVch")
    h_pk8 = nc.inline_tensor(PK8, "cPK8")
    h_pk40 = nc.inline_tensor(PK40, "cPK40")
    h_pk40b = nc.inline_tensor(PK40B, "cPK40B")
    h_wf2bd = nc.inline_tensor(WF2BD, "cWF2BD")
    h_bf1w = nc.inline_tensor(BF1W, "cBF1W")
    h_sel2w = nc.inline_tensor(SEL2W, "cSEL2W")
    h_pk8b = nc.inline_tensor(PK8B, "cPK8B")
    h_pk16 = nc.inline_tensor(PK16, "cPK16")
    h_wf2b = nc.inline_tensor(WF2B, "cWF2B")
    h_one18 = nc.inline_tensor(np.ones((1, 8), np.float32), "cOne18")
    h_ones128 = nc.inline_tensor(
        np.ones((128, 8), np.float32).astype(NPBF), "cOnes128")

    with tile.TileContext(nc) as tc:
        with (
            tc.tile_pool(name="consts", bufs=1) as csts,
            tc.tile_pool(name="xin", bufs=3) as xin,
            tc.tile_pool(name="pooltmp", bufs=2) as ptmp,
            tc.tile_pool(name="mid", bufs=1) as mid,
            tc.tile_pool(name="et", bufs=4) as etp,
            tc.tile_pool(name="outp", bufs=4) as outp,
            tc.tile_pool(name="psA", bufs=2, space="PSUM") as psA,
            tc.tile_pool(name="psT", bufs=1, space="PSUM") as psT,
            tc.tile_pool(name="psAcc", bufs=2, space="PSUM") as psAcc,
            tc.tile_pool(name="psC", bufs=3, space="PSUM") as psC,
        ):
            # ---------------- consts ----------------
            w1t = csts.tile([128, 3, 9, 8], BF16)
            nc.gpsimd.dma_start(out=w1t, in_=h_w1[:])
            w9t = csts.tile([72, 9, 3, 128], BF16)
            nc.gpsimd.dma_start(out=w9t, in_=h_w9[:])
            gsumt = csts.tile([128, 3, 32], F32)
            nc.gpsimd.dma_start(out=gsumt, in_=h_gsum[:])
            gtt = csts.tile([32, 3, 128], F32)
            nc.gpsimd.dma_start(out=gtt, in_=h_gt[:])
            vcht = csts.tile([128, 3, 3], F32)
            nc.gpsimd.dma_start(out=vcht, in_=h_vch[:])
            pk8t = csts.tile([8, PK8.shape[1]], F32)
            nc.gpsimd.dma_start(out=pk8t, in_=h_pk8[:])
            pk40t = csts.tile([40, PK40.shape[1]], F32)
            nc.gpsimd.dma_start(out=pk40t, in_=h_pk40[:])
            pk40bt = csts.tile([40, PK40B.shape[1]], BF16)
            nc.gpsimd.dma_start(out=pk40bt, in_=h_pk40b[:])
            wf2bdt = csts.tile([48, 40], BF16)
            nc.gpsimd.dma_start(out=wf2bdt, in_=h_wf2bd[:])
            bf1wt = csts.tile([48, 1], F32)
            nc.gpsimd.dma_start(out=bf1wt, in_=h_bf1w[:])
            sel2wt = csts.tile([2, 40], BF16)
            nc.gpsimd.dma_start(out=sel2wt, in_=h_sel2w[:])
            pk8bt = csts.tile([8, PK8B.shape[1]], BF16)
            nc.gpsimd.dma_start(out=pk8bt, in_=h_pk8b[:])
            pk16t = csts.tile([16, 9], F32)
            nc.gpsimd.dma_start(out=pk16t, in_=h_pk16[:])
            wf2bt = csts.tile([16, 8], BF16)
            nc.gpsimd.dma_start(out=wf2bt, in_=h_wf2b[:])
            one18t = csts.tile([1, 8], F32)
            nc.gpsimd.dma_start(out=one18t, in_=h_one18[:])
            ones8x = csts.tile([128, 8], BF16)
            nc.gpsimd.dma_start(out=ones8x, in_=h_ones128[:])

            def c8(name):
                c0, w = cols8[name]
                return pk8t[:, c0:c0 + w]

            def c40(name):
                c0, w = cols40[name]
                return pk40t[:, c0:c0 + w]

            def c40b(name):
                c0, w = cols40b[name]
                return pk40bt[:, c0:c0 + w]

            def c8b(name):
                c0, w = cols8b[name]
                return pk8bt[:, c0:c0 + w]

            bf1 = pk16t[:, 8:9]

            eps1 = csts.tile([32, 1], F32)
            nc.vector.memset(eps1, 256.0 * EPS)
            eps40 = csts.tile([40, 1], F32)
            nc.vector.memset(eps40, EPS)
            eps2p = csts.tile([2, 1], F32)
            nc.vector.memset(eps2p, EPS)

            # ---------------- state ----------------
            z0 = mid.tile([128, 3, B, N], F32)
            nc.vector.memset(z0[64:128, 2, :, :], 0.0)
            stat2 = mid.tile([128, 3, B, 2], F32)
            ab = mid.tile([32, 2 * B], F32)
            sbias = mid.tile([128, 3, B, 2], F32)
            zcw = mid.tile([40, N], F32)
            znaw = mid.tile([40, N], BF16)
            qsbw = mid.tile([40, N], BF16)
            ksbw = mid.tile([40, N], BF16)
            vsbw = mid.tile([40, N], BF16)
            qblkw = mid.tile([40, 2, 2, N], BF16)
            vT = mid.tile([128, 2, B, 8], BF16)
            z1w = mid.tile([40, N], F32)
            z2w = mid.tile([40, N], F32)
            pad1 = mid.tile([128, 3, B, 18, 18], BF16)
            spadw = mid.tile([40, 18, 18], BF16)
            sp9 = mid.tile([72, B, N], BF16)

            # ---------------- phase 1: load + pool ----------------
            def pool(xt, dst):
                wp = ptmp.tile([128, 1024], F32, tag="wp")
                for hh in range(2):
                    nc.vector.reduce_sum(
                        out=wp[:, 512 * hh:512 * hh + 512],
                        in_=xt[:, 2048 * hh:2048 * hh + 2048].rearrange(
                            "p (a b) -> p a b", b=4),
                        axis=AX.X)
                wpv = wp.rearrange("p (hb hi wb) -> p hb hi wb", hi=4, wb=16)
                t01 = ptmp.tile([128, 16, 16], F32, tag="t01")
                nc.gpsimd.tensor_add(t01, wpv[:, :, 0, :], wpv[:, :, 1, :])
                t23 = ptmp.tile([128, 16, 16], F32, tag="t23")
                nc.gpsimd.tensor_add(t23, wpv[:, :, 2, :], wpv[:, :, 3, :])
                nc.gpsimd.tensor_add(dst, t01, t23)

            nc.gpsimd.memset(pad1, 0.0)
            loads = [(0, 0), (0, 1), (None, 2), (1, 0), (1, 1)]
            z0c2 = ptmp.tile([128, N], F32, tag="z0c2")
            with nc.named_scope("pool"):
                for b, k in loads:
                    xt = xin.tile([128, 4096], F32, tag="xt")
                    if b is not None:
                        src_ap = x[b, 128 * k:128 * (k + 1)].rearrange(
                            "c h w -> c (h w)")
                        nc.sync.dma_start(out=xt[:, 0:2048], in_=src_ap[:, 0:2048])
                        nc.scalar.dma_start(out=xt[:, 2048:4096],
                                            in_=src_ap[:, 2048:4096])
                        pool(xt, z0[:, k, b, :])
                    else:
                        for bb in range(2):
                            src_ap = x[bb, 256:320].rearrange("c h w -> c (h w)")
                            eng = nc.sync if bb == 0 else nc.scalar
                            eng.dma_start(out=xt[64 * bb:64 * bb + 64, :],
                                          in_=src_ap)
                        pool(xt, z0c2)
                        nc.gpsimd.dma_start(out=z0[0:64, 2, 0, :],
                                            in_=z0c2[0:64, :])
                        nc.gpsimd.dma_start(out=z0[0:64, 2, 1, :],
                                            in_=z0c2[64:128, :])

            # ---------------- middle: one batch-stacked chain ----------------
            with nc.named_scope("middle"):
                # GN1 + conv1 per batch: batch 0's section overlaps
                # batch 1's input DMA (engine streams execute in order)
                pzw = psA.tile([40, N], F32, tag="ps", name="pzw")
                nc.vector.memset(pzw, 0.0)
                for b in range(B):
                    for k in range(3):
                        st6 = ptmp.tile([128, 6], F32, tag="st6")
                        nc.vector.bn_stats(out=st6, in_=z0[:, k, b, :])
                        nc.vector.bn_aggr(out=stat2[:, k, b, :], in_=st6)
                        tm = ptmp.tile([128, 1], F32, tag="tm")
                        nc.vector.tensor_mul(tm, stat2[:, k, b, 0:1],
                                             stat2[:, k, b, 0:1])
                        nc.vector.tensor_add(stat2[:, k, b, 1:2],
                                             stat2[:, k, b, 1:2], tm)
                    pg = psA.tile([32, 2], F32, tag="ps", name=f"pg{b}")
                    for k in range(3):
                        nc.tensor.matmul(pg, gsumt[:, k, :], stat2[:, k, b, :],
                                         start=(k == 0), stop=(k == 2))
                    gm = ptmp.tile([32, 2], F32, tag="gm")
                    nc.vector.tensor_copy(gm, pg)
                    gv = ptmp.tile([32, 1], F32, tag="gv")
                    nc.vector.tensor_mul(gv, gm[:, 0:1], gm[:, 0:1])
                    nc.vector.tensor_sub(gv, gm[:, 1:2], gv)
                    nc.scalar.activation(out=gv, in_=gv, func=AF.Sqrt,
                                         bias=eps1)
                    nc.vector.reciprocal(out=ab[:, 2 * b:2 * b + 1], in_=gv)
                    nc.vector.tensor_copy(ab[:, 2 * b + 1:2 * b + 2],
                                          gm[:, 0:1])
                    pbc = psA.tile([128, 3, 2], F32, tag="ps", name=f"pbc{b}")
                    for k in range(3):
                        nc.tensor.matmul(pbc[:, k, :], gtt[:, k, :],
                                         ab[:, 2 * b:2 * b + 2],
                                         start=True, stop=True)
                    g1b = vcht[:, :, 0].unsqueeze(2)
                    b1b = vcht[:, :, 1].unsqueeze(2)
                    nc.vector.tensor_mul(
                        sbias[:, :, b, 0:1], pbc[:, :, 0:1], g1b)
                    tm2 = ptmp.tile([128, 3, 1], F32, tag="tm2")
                    nc.vector.tensor_mul(tm2, sbias[:, :, b, 0:1],
                                         pbc[:, :, 1:2])
                    nc.vector.tensor_sub(sbias[:, :, b, 1:2], b1b, tm2)
                    for k in range(3):
                        nc.scalar.activation(
                            out=pad1[:, k, b, 1:17, 1:17],
                            in_=z0[:, k, b, :].rearrange("p (h w) -> p h w",
                                                         w=16),
                            func=AF.Silu,
                            scale=sbias[:, k, b, 0:1],
                            bias=sbias[:, k, b, 1:2])
                    first = True
                    for k in range(3):
                        for ky in range(3):
                            for kx in range(3):
                                nc.tensor.matmul(
                                    pzw[32 * b:32 * b + 8, :],
                                    w1t[:, k, 3 * ky + kx, :],
                                    pad1[:, k, b, ky:ky + 16, kx:kx + 16],
                                    start=first,
                                    stop=(k == 2 and ky == 2 and kx == 2))
                                first = False
                nc.vector.tensor_scalar_add(out=zcw, in0=pzw,
                                            scalar1=c40("b_conv1"))

                # ---- attention (wide layout) ----
                st6a = ptmp.tile([40, 6], F32, tag="st6a")
                nc.vector.bn_stats(out=st6a, in_=zcw)
                mva = ptmp.tile([40, 2], F32, tag="mva")
                nc.vector.bn_aggr(out=mva, in_=st6a)
                ra = ptmp.tile([40, 1], F32, tag="ra")
                nc.scalar.activation(out=ra, in_=mva[:, 1:2], func=AF.Sqrt,
                                     bias=eps40)
                nc.vector.reciprocal(out=ra, in_=ra)
                sca = ptmp.tile([40, 2], F32, tag="sca")
                nc.vector.tensor_mul(sca[:, 0:1], ra, c40("ga"))
                tm3 = ptmp.tile([40, 1], F32, tag="tm3")
                nc.vector.tensor_mul(tm3, sca[:, 0:1], mva[:, 0:1])
                nc.vector.tensor_scalar(
                    out=sca[:, 1:2], in0=tm3, scalar1=c40("ba"), scalar2=-1.0,
                    op0=ALU.subtract, op1=ALU.mult)
                nc.vector.tensor_scalar(
                    out=znaw, in0=zcw, scalar1=sca[:, 0:1], scalar2=sca[:, 1:2],
                    op0=ALU.mult, op1=ALU.add)
                for wname, bname, dst in [("wqBD", "bq8", qsbw),
                                          ("wkBD", "bk8", ksbw),
                                          ("wvBD", "bv", vsbw)]:
                    pqkv = psA.tile([40, N], F32, tag="ps",
                                    name=f"pqkv_{wname}")
                    nc.tensor.matmul(pqkv, c40b(wname), znaw,
                                     start=True, stop=True)
                    nc.vector.tensor_scalar_add(out=dst, in0=pqkv,
                                                scalar1=c40(bname))
                qmt = c40b("qmaskW").rearrange("p (a c) -> p a c", a=2)
                nc.vector.tensor_mul(
                    qblkw,
                    qsbw.unsqueeze(1).unsqueeze(1).broadcast_to([40, 2, 2, N]),
                    qmt.unsqueeze(3).broadcast_to([40, 2, 2, N]))
                for b in range(B):
                    for mc in range(2):
                        pvt = psT.tile([128, 2, N], F32, tag="pt")
                        nc.tensor.matmul(
                            pvt[:, 0, 0:8],
                            vsbw[32 * b:32 * b + 8, 128 * mc:128 * (mc + 1)],
                            c40b("identW")[32 * b:32 * b + 8, :],
                            start=True, stop=True)
                        nc.vector.tensor_copy(vT[:, mc, b, :], pvt[:, 0, 0:8])
                wpH_t = c40b("wpHW").rearrange("p (a c) -> p a c", a=4)
                ppw = psA.tile([40, N], F32, tag="ps")
                nc.vector.memset(ppw, 0.0)
                for blk in range(2):
                    psum_s = psAcc.tile([40, 2, N], F32, tag="acc",
                                        name=f"psum_s{blk}")
                    nc.vector.memset(psum_s, 1.0)
                    psum_e = psAcc.tile([40, 2, N], F32, tag="acc",
                                        name=f"psum_e{blk}")
                    nc.vector.memset(psum_e, 0.0)
                    for b in range(B):
                        for mc in range(2):
                            pst = psT.tile([128, 2, N], F32, tag="pt")
                            nc.tensor.matmul(
                                pst.rearrange("p a n -> p (a n)"),
                                ksbw[32 * b:32 * b + 8,
                                     128 * mc:128 * (mc + 1)],
                                qblkw[32 * b:32 * b + 8, blk].rearrange(
                                    "p a n -> p (a n)"),
                                start=True, stop=True)
                            et = etp.tile([128, 2, N], BF16, tag="et")
                            nc.scalar.activation(out=et, in_=pst, func=AF.Exp)
                            etf = et.rearrange("p a n -> p (a n)")
                            nc.tensor.matmul(
                                psum_s[32 * b:32 * b + 8, :, :].rearrange(
                                    "p a n -> p (a n)"),
                                ones8x, etf, start=(mc == 0), stop=(mc == 1))
                            nc.tensor.matmul(
                                psum_e[32 * b:32 * b + 8, :, :].rearrange(
                                    "p a n -> p (a n)"),
                                vT[:, mc, b, :], etf,
                                start=(mc == 0), stop=(mc == 1))
                    den = ptmp.tile([40, 2, N], F32, tag="den")
                    nc.vector.reciprocal_approx_fast(out=den, in_=psum_s)
                    aoblk = ptmp.tile([40, 2, N], BF16, tag="aoblk")
                    nc.vector.tensor_mul(aoblk, psum_e, den)
                    # cross-head lanes: finite garbage x zero proj weight
                    for b in range(B):
                        for hp in range(2):
                            nc.tensor.matmul(
                                ppw[32 * b:32 * b + 8, :],
                                wpH_t[32 * b:32 * b + 8, 2 * blk + hp, :],
                                aoblk[32 * b:32 * b + 8, hp, :],
                                start=(blk == 0 and hp == 0),
                                stop=(blk == 1 and hp == 1))
                nc.vector.tensor_scalar_add(out=z1w, in0=ppw,
                                            scalar1=c40("b_proj"))
                nc.vector.tensor_add(z1w, z1w, zcw)

                # ---- per-pixel LN + MLP (wide) ----
                z1b = ptmp.tile([40, N], BF16, tag="z1b")
                nc.vector.tensor_copy(z1b, z1w)
                sq8 = ptmp.tile([40, N], BF16, tag="sq8")
                nc.vector.tensor_mul(sq8, z1b, z1b)
                plnA = psA.tile([2, N], F32, tag="ps", name="plnA")
                nc.tensor.matmul(plnA, c40b("w8BD"), z1b, start=True, stop=True)
                plnB = psA.tile([2, N], F32, tag="ps", name="plnB")
                nc.tensor.matmul(plnB, c40b("w8BD"), sq8, start=True, stop=True)
                muF = ptmp.tile([2, N], F32, tag="muF")
                nc.vector.tensor_copy(muF, plnA)
                muS = ptmp.tile([2, N], BF16, tag="muS")
                nc.vector.tensor_copy(muS, muF)
                musq = ptmp.tile([2, N], F32, tag="musq")
                nc.vector.tensor_mul(musq, muF, muF)
                rsS = ptmp.tile([2, N], F32, tag="rsS")
                nc.vector.tensor_sub(rsS, plnB, musq)
                nc.scalar.activation(out=rsS, in_=rsS, func=AF.Sqrt, bias=eps2p)
                rsF = ptmp.tile([2, N], F32, tag="rsF")
                nc.vector.reciprocal_approx_fast(out=rsF, in_=rsS)
                rsB = ptmp.tile([2, N], BF16, tag="rsB")
                nc.vector.tensor_copy(rsB, rsF)
                pbrM = psA.tile([40, N], F32, tag="ps", name="pbrM")
                nc.tensor.matmul(pbrM, sel2wt, muS, start=True, stop=True)
                pbrR = psA.tile([40, N], F32, tag="ps", name="pbrR")
                nc.tensor.matmul(pbrR, sel2wt, rsB, start=True, stop=True)
                cen = ptmp.tile([40, N], F32, tag="cen")
                nc.vector.tensor_sub(cen, z1w, pbrM)
                nc.vector.tensor_mul(cen, cen, pbrR)
                lnt = ptmp.tile([40, N], BF16, tag="lnt")
                nc.vector.tensor_scalar(
                    out=lnt, in0=cen, scalar1=c40("ln_g"), scalar2=c40("ln_b"),
                    op0=ALU.mult, op1=ALU.add)
                pf1 = psA.tile([48, N], F32, tag="ps")
                nc.tensor.matmul(pf1, c40b("wf1BD"), lnt, start=True, stop=True)
                hmid = ptmp.tile([48, N], BF16, tag="hmid")
                nc.scalar.activation(out=hmid, in_=pf1, func=AF.Gelu, bias=bf1wt)
                pf2 = psA.tile([40, N], F32, tag="ps")
                nc.tensor.matmul(pf2, wf2bdt, hmid, start=True, stop=True)
                nc.vector.tensor_scalar_add(out=z2w, in0=pf2,
                                            scalar1=c40("b_fc2"))
                nc.vector.tensor_add(z2w, z2w, z1w)

                # ---- GN2 + SiLU into padded tile (wide) ----
                st6b = ptmp.tile([40, 6], F32, tag="st6b")
                nc.vector.bn_stats(out=st6b, in_=z2w)
                mvb = ptmp.tile([40, 2], F32, tag="mvb")
                nc.vector.bn_aggr(out=mvb, in_=st6b)
                rb2 = ptmp.tile([40, 1], F32, tag="rb2")
                nc.scalar.activation(out=rb2, in_=mvb[:, 1:2], func=AF.Sqrt,
                                     bias=eps40)
                nc.vector.reciprocal(out=rb2, in_=rb2)
                scb = ptmp.tile([40, 2], F32, tag="scb")
                nc.vector.tensor_mul(scb[:, 0:1], rb2, c40("g2"))
                tm4 = ptmp.tile([40, 1], F32, tag="tm4")
                nc.vector.tensor_mul(tm4, scb[:, 0:1], mvb[:, 0:1])
                nc.vector.tensor_scalar(
                    out=scb[:, 1:2], in0=tm4, scalar1=c40("b2"), scalar2=-1.0,
                    op0=ALU.subtract, op1=ALU.mult)
                nc.gpsimd.memset(spadw, 0.0)
                nc.scalar.activation(
                    out=spadw[:, 1:17, 1:17],
                    in_=z2w.rearrange("p (h w) -> p h w", w=16),
                    func=AF.Silu, scale=scb[:, 0:1], bias=scb[:, 1:2])
                for dy in range(3):
                    for dx in range(3):
                        slot = 3 * dy + dx
                        for b in range(B):
                            eng = nc.sync if (slot + b) % 2 == 0 else nc.scalar
                            eng.dma_start(
                                out=sp9[8 * slot:8 * slot + 8, b, :],
                                in_=spadw[32 * b:32 * b + 8,
                                          dy:dy + 16, dx:dx + 16])

            # ---------------- conv2 + interleave + out ----------------
            sp9f = sp9.rearrange("p a b -> p (a b)")
            with nc.named_scope("conv2"):
                for c in range(3):
                    if c < 2:
                        ots = [outp.tile([128, 64, 64], F32, tag="oc",
                                         name=f"oc_b{bb}c{c}")
                               for bb in range(B)]
                    else:
                        shared = outp.tile([128, 64, 64], F32, tag="oc",
                                           name="oc_c2")
                        ots = [shared, shared]
                    for g, (pi, pj) in enumerate(GROUPS):
                        r0, nr = ROWSETS[pi]
                        c0, ncc = ROWSETS[pj]
                        pcv = psC.tile([128, B, 16, 16], F32, tag="pcv")
                        nc.tensor.matmul(
                            pcv.rearrange("p a b c -> p (a b c)"),
                            w9t[:, g, c, :], sp9f, start=True, stop=True)
                        ncopy = 0
                        for b in range(B):
                            if c < 2:
                                p0, pn = 0, 128
                            else:
                                p0, pn = 64 * b, 64
                            src_b = pcv[p0:p0 + pn, b].unsqueeze(3).broadcast_to(
                                [pn, 16, 16, ncc])
                            base5 = ots[b].rearrange(
                                "p (bi ri) (bj rj) -> p bi ri bj rj",
                                ri=4, rj=4)
                            bias_ap = vcht[p0:p0 + pn, c, 2:3]
                            for rr in range(nr):
                                dst = base5[p0:p0 + pn, :, r0 + rr, :,
                                            c0:c0 + ncc]
                                if ncopy % 2 == 0:
                                    nc.vector.tensor_scalar_add(
                                        out=dst, in0=src_b, scalar1=bias_ap)
                                else:
                                    nc.scalar.activation(
                                        out=dst, in_=src_b, func=AF.Identity,
                                        bias=bias_ap)
                                ncopy += 1
                    for b in range(B):
                        if c < 2:
                            dstd = out[b, 128 * c:128 * (c + 1)].rearrange(
                                "c h w -> c (h w)")
                            st = ots[b].rearrange("p h w -> p (h w)")
                            nc.sync.dma_start(out=dstd[:, 0:2048],
                                              in_=st[:, 0:2048])
                            nc.scalar.dma_start(out=dstd[:, 2048:4096],
                                                in_=st[:, 2048:4096])
                        else:
                            p0 = 64 * b
                            dstd = out[b, 256:320].rearrange("c h w -> c (h w)")
                            st = shared.rearrange("p h w -> p (h w)")
                            nc.sync.dma_start(out=dstd[:, 0:2048],
                                              in_=st[p0:p0 + 64, 0:2048])
                            nc.scalar.dma_start(out=dstd[:, 2048:4096],
                                                in_=st[p0:p0 + 64, 2048:4096])
    nc.compile()
    return nc


_cache = {}


def kernel(**inputs):
    x = np.ascontiguousarray(np.asarray(inputs["x"], np.float32))
    params = {k: np.asarray(v, np.float32) for k, v in inputs.items()
              if k != "x"}

    key = hash(tuple(sorted((k, v.tobytes()) for k, v in params.items())))
    if key not in _cache:
        _cache[key] = build(params)
    nc = _cache[key]

    in_maps = [{"x": np.ascontiguousarray(x[B * i:B * (i + 1)])}
               for i in range(NCORES)]
    res = run_bass_kernel_spmd(nc, in_maps, core_ids=list(range(NCORES)),
                               trace=KERNEL_TRACE)
    out = np.concatenate([res.results[i]["out"] for i in range(NCORES)], axis=0)
    if KERNEL_TRACE:
        kernel.last_result = res
    return out


# revision 28
# speedup vs baseline: 1.0049x; 1.0049x over previous
"""Trainium2 Bass kernel for the nn_Adaptor problem.

Computation (per batch image):
  avgpool4x4 -> GN(32 groups)+SiLU -> conv3x3 320->8 -> attention(4 heads) ->
  per-pixel LN + MLP -> GN(8)+SiLU -> upsample x4 nearest -> conv3x3 8->320

Distribution: pure data parallel over batch. 16 images / 8 cores = 2 per core.
Params are baked into the NEFF as inline consts (recomputed from the numpy
arrays passed to kernel() at trace time).

Implementation notes:
  - pooling keeps raw 4x4 sums (16x scale); GN1 uses eps_eff = 256*eps so the
    normalized output is exact.
  - GN1 group stats via per-channel bn_stats + grouping-matrix matmuls on PE.
  - All norm+SiLU applications fused into single scalar-engine activations.
  - conv1 as 9 shifted-window matmuls over a zero-padded 18x18 tile; both
    local batch images stacked along the matmul free dim (N=512).
  - attention: transposed scores E^T = exp(k^T q) without max subtraction
    (|scores| < 0.5 for this operator family); two heads per matmul via
    zero-masked q blocks; softmax denominators via ones-matmul column sums;
    head gather folded into zero-masked projection matmuls.
  - GN2 applied pre-upsample (nearest-upsample preserves per-channel stats).
  - conv2-after-upsample collapses to 9 phase groups with collapsed weights on
    the 16x16 grid (K=72 over a 9-slot shifted-window stack); phase outputs are
    interleaved on-chip by strided copies with step-0 column duplication, then
    written out with fully contiguous DMAs split across both HWDGE rings.
  - the middle is one batch-stacked dependency chain; engine streams execute
    in order, so fewer/wider ops beat two interleaved per-batch chains.
"""

import ml_dtypes
import numpy as np

import concourse.bass as bass
import concourse.bacc as bacc
import concourse.tile as tile
from concourse import mybir
from concourse.bass_utils import run_bass_kernel_spmd

F32 = mybir.dt.float32
BF16 = mybir.dt.bfloat16
NPBF = ml_dtypes.bfloat16
AF = mybir.ActivationFunctionType
ALU = mybir.AluOpType
AX = mybir.AxisListType

CH, C, D, HEADS = 320, 4, 8, 4
EPS = 1e-5
B = 2
N = 256
NCORES = 8

TAPSETS = {
    0: [(0, (0,)), (1, (1, 2))],
    1: [(1, (0, 1, 2))],
    3: [(1, (0, 1)), (2, (2,))],
}
GROUPS = [(pi, pj) for pi in (0, 1, 3) for pj in (0, 1, 3)]
ROWSETS = {0: (0, 1), 1: (1, 2), 3: (3, 1)}   # (start row, duplication count)

KERNEL_TRACE = False


def _conv2_tables(w_conv2):
    """W9 [72, 9, 3, 128]: collapsed per-phase-group weights over the 9-slot
    shifted-window stack; chunk 2 duplicated into cols 64:128 (two-batch
    chunk-2 matmul keeps batch 1 at psum partitions 64:128)."""
    W9 = np.zeros((72, 9, 3, 128), np.float32)
    for g, (pi, pj) in enumerate(GROUPS):
        for (dy, kys) in TAPSETS[pi]:
            for (dx, kxs) in TAPSETS[pj]:
                s = 3 * dy + dx
                wsum = np.zeros((CH, D), np.float32)
                for ky in kys:
                    for kx in kxs:
                        wsum += w_conv2[:, :, ky, kx]
                for c in range(3):
                    oc0 = 128 * c
                    ocn = min(128, CH - oc0)
                    blk = wsum[oc0:oc0 + ocn].T
                    W9[8 * s:8 * s + 8, g, c, :ocn] += blk
                    if c == 2:
                        W9[8 * s:8 * s + 8, g, c, 64:64 + ocn] += blk
    return W9


def build(params):
    P = params
    nc = bacc.Bacc("TRN2")

    x = nc.dram_tensor("x", [B, CH, 64, 64], F32, kind="ExternalInput")
    out = nc.dram_tensor("out", [B, CH, 64, 64], F32, kind="ExternalOutput")

    # ---------------- host-side constant prep ----------------
    s = float(1 / np.sqrt(D // HEADS))
    wq = P["w_qkv"].copy()
    bq = P["b_qkv"].copy()
    wq[D:2 * D] *= s
    bq[D:2 * D] *= s

    W1 = np.zeros((128, 3, 9, 8), np.float32)
    for c in range(3):
        c0 = 128 * c
        cn = min(128, CH - c0)
        for ky in range(3):
            for kx in range(3):
                W1[:cn, c, 3 * ky + kx, :] = P["w_conv1"][:, c0:c0 + cn, ky, kx].T

    W9 = _conv2_tables(P["w_conv2"])

    Gsum = np.zeros((128, 3, 32), np.float32)
    GT = np.zeros((32, 3, 128), np.float32)
    for c in range(CH):
        k, p = divmod(c, 128)
        Gsum[p, k, c // 10] = 0.1
        GT[c // 10, k, p] = 1.0

    def chunks(v, dup2=False):
        a = np.zeros((128, 3), np.float32)
        for c in range(3):
            c0 = 128 * c
            cn = min(128, CH - c0)
            a[:cn, c] = v[c0:c0 + cn]
            if dup2 and c == 2:
                a[64:64 + cn, c] = v[c0:c0 + cn]
        return a

    vch = np.stack([chunks(P["g1"]), chunks(P["b1"]),
                    chunks(P["b_conv2"], dup2=True)], axis=2)  # [128, 3, 3]

    cols8, pk8 = {}, []

    def pack8(name, arr):
        arr = np.asarray(arr, np.float32).reshape(8, -1)
        cols8[name] = (sum(a.shape[1] for a in pk8), arr.shape[1])
        pk8.append(arr)

    pack8("ones8", np.full((8, 1), 0.125, np.float32))
    for nm, val in [("bq8", bq[0:D]), ("bk8", bq[D:2 * D]), ("bv", bq[2 * D:]),
                    ("b_conv1", P["b_conv1"]), ("b_proj", P["b_proj"]),
                    ("ln_g", P["ln_g"]), ("ln_b", P["ln_b"]),
                    ("b_fc2", P["b_fc2"]), ("ga", P["ga"]), ("ba", P["ba"]),
                    ("g2", P["g2"]), ("b2", P["b2"])]:
        pack8(nm, val.reshape(8, 1))
    PK8 = np.concatenate(pk8, axis=1)

    # wide-middle consts: batch b lives at partition base 32*b
    def widen(v8, n=40):
        a = np.zeros((n, 1), np.float32)
        a[0:8, 0] = v8
        a[32:40, 0] = v8
        return a

    def blockdiag(w, n_in=40, n_out=40):
        a = np.zeros((n_in, n_out), np.float32)
        r, c = w.shape
        a[0:r, 0:c] = w
        a[32:32 + r, 32:32 + c] = w
        return a

    cols40, pk40 = {}, []

    def pack40(name, arr):
        arr = np.asarray(arr, np.float32).reshape(40, -1)
        cols40[name] = (sum(a.shape[1] for a in pk40), arr.shape[1])
        pk40.append(arr)

    for nm, val in [("bq8", bq[0:D]), ("bk8", bq[D:2 * D]), ("bv", bq[2 * D:]),
                    ("b_conv1", P["b_conv1"]), ("b_proj", P["b_proj"]),
                    ("ln_g", P["ln_g"]), ("ln_b", P["ln_b"]),
                    ("b_fc2", P["b_fc2"]), ("ga", P["ga"]), ("ba", P["ba"]),
                    ("g2", P["g2"]), ("b2", P["b2"])]:
        pack40(nm, widen(val.reshape(8)))
    PK40 = np.concatenate(pk40, axis=1)

    cols40b, pk40b = {}, []

    def pack40b(name, arr):
        arr = np.asarray(arr, np.float32)
        arr = arr.reshape(40, -1)
        cols40b[name] = (sum(a.shape[1] for a in pk40b), arr.shape[1])
        pk40b.append(arr)

    pack40b("wqBD", blockdiag(wq[0:D].T))
    pack40b("wkBD", blockdiag(wq[D:2 * D].T))
    pack40b("wvBD", blockdiag(P["w_qkv"][2 * D:3 * D].T))
    identW = np.zeros((40, 8), np.float32)
    identW[0:8] = np.eye(8)
    identW[32:40] = np.eye(8)
    pack40b("identW", identW)
    qmaskW = np.zeros((40, 4), np.float32)
    for c in range(8):
        qmaskW[c, c // 2] = 1.0
        qmaskW[32 + c, c // 2] = 1.0
    pack40b("qmaskW", qmaskW)
    wpHW = np.zeros((40, 4, 8), np.float32)
    for c in range(8):
        wpHW[c, c // 2, :] = P["w_proj"][:, c]
        wpHW[32 + c, c // 2, :] = P["w_proj"][:, c]
    pack40b("wpHW", wpHW.reshape(40, 32))
    pack40b("wf1BD", blockdiag(P["w_fc1"].T, 40, 48))
    w8BD = np.zeros((40, 2), np.float32)
    w8BD[0:8, 0] = 0.125
    w8BD[32:40, 1] = 0.125
    pack40b("w8BD", w8BD)
    PK40B = np.concatenate(pk40b, axis=1).astype(NPBF)

    WF2BD = blockdiag(P["w_fc2"].T, 48, 40).astype(NPBF)   # [48, 40]
    BF1W = np.zeros((48, 1), np.float32)
    BF1W[0:16, 0] = P["b_fc1"]
    BF1W[32:48, 0] = P["b_fc1"]
    SEL2W = np.zeros((2, 40), np.float32)
    SEL2W[0, 0:8] = 1.0
    SEL2W[1, 32:40] = 1.0
    SEL2W = SEL2W.astype(NPBF)

    cols8b, pk8b = {}, []

    def pack8b(name, arr):
        arr = np.asarray(arr, np.float32).reshape(8, -1)
        cols8b[name] = (sum(a.shape[1] for a in pk8b), arr.shape[1])
        pk8b.append(arr)

    pack8b("wqT", wq[0:D].T)
    pack8b("wkT", wq[D:2 * D].T)
    pack8b("wvT", P["w_qkv"][2 * D:3 * D].T)
    pack8b("ident8", np.eye(8, dtype=np.float32))
    qmask = np.zeros((8, 4), np.float32)
    for c in range(8):
        qmask[c, c // 2] = 1.0
    pack8b("qmask", qmask)
    wpH = np.zeros((8, 4, 8), np.float32)
    for c in range(8):
        wpH[c, c // 2, :] = P["w_proj"][:, c]
    pack8b("wpH", wpH.reshape(8, 32))
    pack8b("wf1T", P["w_fc1"].T)
    PK8B = np.concatenate(pk8b, axis=1).astype(NPBF)

    PK16 = np.concatenate([P["w_fc2"].T, P["b_fc1"].reshape(16, 1)], axis=1)
    WF2B = P["w_fc2"].T.astype(NPBF)

    h_w1 = nc.inline_tensor(W1.astype(NPBF), "cW1")
    h_w9 = nc.inline_tensor(W9.astype(NPBF), "cW9")
    h_gsum = nc.inline_tensor(Gsum, "cGsum")
    h_gt = nc.inline_tensor(GT, "cGT")
    h_vch = nc.inline_tensor(vch, "cVch")
    h_pk8 = nc.inline_tensor(PK8, "cPK8")
    h_pk40 = nc.inline_tensor(PK40, "cPK40")
    h_pk40b = nc.inline_tensor(PK40B, "cPK40B")
    h_wf2bd = nc.inline_tensor(WF2BD, "cWF2BD")
    h_bf1w = nc.inline_tensor(BF1W, "cBF1W")
    h_sel2w = nc.inline_tensor(SEL2W, "cSEL2W")
    h_pk8b = nc.inline_tensor(PK8B, "cPK8B")
    h_pk16 = nc.inline_tensor(PK16, "cPK16")
    h_wf2b = nc.inline_tensor(WF2B, "cWF2B")
    h_one18 = nc.inline_tensor(np.ones((1, 8), np.float32), "cOne18")
    h_ones128 = nc.inline_tensor(
        np.ones((128, 8), np.float32).astype(NPBF), "cOnes128")

    with tile.TileContext(nc) as tc:
        with (
            tc.tile_pool(name="consts", bufs=1) as csts,
            tc.tile_pool(name="xin", bufs=3) as xin,
            tc.tile_pool(name="pooltmp", bufs=2) as ptmp,
            tc.tile_pool(name="mid", bufs=1) as mid,
            tc.tile_pool(name="et", bufs=4) as etp,
            tc.tile_pool(name="outp", bufs=4) as outp,
            tc.tile_pool(name="psA", bufs=2, space="PSUM") as psA,
            tc.tile_pool(name="psT", bufs=1, space="PSUM") as psT,
            tc.tile_pool(name="psAcc", bufs=2, space="PSUM") as psAcc,
            tc.tile_pool(name="psC", bufs=3, space="PSUM") as psC,
        ):
            # ---------------- consts ----------------
            w1t = csts.tile([128, 3, 9, 8], BF16)
            nc.gpsimd.dma_start(out=w1t, in_=h_w1[:])
            w9t = csts.tile([72, 9, 3, 128], BF16)
            nc.gpsimd.dma_start(out=w9t, in_=h_w9[:])
            gsumt = csts.tile([128, 3, 32], F32)
            nc.gpsimd.dma_start(out=gsumt, in_=h_gsum[:])
            gtt = csts.tile([32, 3, 128], F32)
            nc.gpsimd.dma_start(out=gtt, in_=h_gt[:])
            vcht = csts.tile([128, 3, 3], F32)
            nc.gpsimd.dma_start(out=vcht, in_=h_vch[:])
            pk8t = csts.tile([8, PK8.shape[1]], F32)
            nc.gpsimd.dma_start(out=pk8t, in_=h_pk8[:])
            pk40t = csts.tile([40, PK40.shape[1]], F32)
            nc.gpsimd.dma_start(out=pk40t, in_=h_pk40[:])
            pk40bt = csts.tile([40, PK40B.shape[1]], BF16)
            nc.gpsimd.dma_start(out=pk40bt, in_=h_pk40b[:])
            wf2bdt = csts.tile([48, 40], BF16)
            nc.gpsimd.dma_start(out=wf2bdt, in_=h_wf2bd[:])
            bf1wt = csts.tile([48, 1], F32)
            nc.gpsimd.dma_start(out=bf1wt, in_=h_bf1w[:])
            sel2wt = csts.tile([2, 40], BF16)
            nc.gpsimd.dma_start(out=sel2wt, in_=h_sel2w[:])
            pk8bt = csts.tile([8, PK8B.shape[1]], BF16)
            nc.gpsimd.dma_start(out=pk8bt, in_=h_pk8b[:])
            pk16t = csts.tile([16, 9], F32)
            nc.gpsimd.dma_start(out=pk16t, in_=h_pk16[:])
            wf2bt = csts.tile([16, 8], BF16)
            nc.gpsimd.dma_start(out=wf2bt, in_=h_wf2b[:])
            one18t = csts.tile([1, 8], F32)
            nc.gpsimd.dma_start(out=one18t, in_=h_one18[:])
            ones8x = csts.tile([128, 8], BF16)
            nc.gpsimd.dma_start(out=ones8x, in_=h_ones128[:])

            def c8(name):
                c0, w = cols8[name]
                return pk8t[:, c0:c0 + w]

            def c40(name):
                c0, w = cols40[name]
                return pk40t[:, c0:c0 + w]

            def c40b(name):
                c0, w = cols40b[name]
                return pk40bt[:, c0:c0 + w]

            def c8b(name):
                c0, w = cols8b[name]
                return pk8bt[:, c0:c0 + w]

            bf1 = pk16t[:, 8:9]

            eps1 = csts.tile([32, 1], F32)
            nc.vector.memset(eps1, 256.0 * EPS)
            eps40 = csts.tile([40, 1], F32)
            nc.vector.memset(eps40, EPS)
            eps2p = csts.tile([2, 1], F32)
            nc.vector.memset(eps2p, EPS)

            # ---------------- state ----------------
            z0 = mid.tile([128, 3, B, N], F32)
            nc.vector.memset(z0[64:128, 2, :, :], 0.0)
            stat2 = mid.tile([128, 3, B, 2], F32)
            ab = mid.tile([32, 2 * B], F32)
            sbias = mid.tile([128, 3, B, 2], F32)
            zcw = mid.tile([40, N], F32)
            znaw = mid.tile([40, N], BF16)
            qsbw = mid.tile([40, N], BF16)
            ksbw = mid.tile([40, N], BF16)
            vsbw = mid.tile([40, N], BF16)
            qblkw = mid.tile([40, 2, 2, N], BF16)
            vT = mid.tile([128, 2, B, 8], BF16)
            z1w = mid.tile([40, N], F32)
            z2w = mid.tile([40, N], F32)
            pad1 = mid.tile([128, 3, B, 18, 18], BF16)
            spadw = mid.tile([40, 18, 18], BF16)
            sp9 = mid.tile([72, B, N], BF16)

            # ---------------- phase 1: load + pool ----------------
            def pool(xt, dst):
                wp = ptmp.tile([128, 1024], F32, tag="wp")
                for hh in range(2):
                    nc.vector.reduce_sum(
                        out=wp[:, 512 * hh:512 * hh + 512],
                        in_=xt[:, 2048 * hh:2048 * hh + 2048].rearrange(
                            "p (a b) -> p a b", b=4),
                        axis=AX.X)
                wpv = wp.rearrange("p (hb hi wb) -> p hb hi wb", hi=4, wb=16)
                t01 = ptmp.tile([128, 16, 16], F32, tag="t01")
                nc.gpsimd.tensor_add(t01, wpv[:, :, 0, :], wpv[:, :, 1, :])
                t23 = ptmp.tile([128, 16, 16], F32, tag="t23")
                nc.gpsimd.tensor_add(t23, wpv[:, :, 2, :], wpv[:, :, 3, :])
                nc.gpsimd.tensor_add(dst, t01, t23)

            nc.gpsimd.memset(pad1, 0.0)
            # issue every input DMA up front (sequencer streams carry only
            # DMAs, so batch-1 transfers start while batch-0 computes);
            # pool reduces are emitted per batch AFTER that batch's GN1
            # consumers are traced, keeping the Vector stream unblocked.
            loads = [(0, 0), (0, 1), (None, 2), (1, 0), (1, 1)]
            z0c2 = ptmp.tile([128, N], F32, tag="z0c2")
            xts = []
            with nc.named_scope("pool"):
                for i, (b, k) in enumerate(loads):
                    xt = xin.tile([128, 4096], F32, tag="xt", name=f"xt{i}")
                    xts.append(xt)
                    if b is not None:
                        src_ap = x[b, 128 * k:128 * (k + 1)].rearrange(
                            "c h w -> c (h w)")
                        nc.sync.dma_start(out=xt[:, 0:2048],
                                          in_=src_ap[:, 0:2048])
                        nc.scalar.dma_start(out=xt[:, 2048:4096],
                                            in_=src_ap[:, 2048:4096])
                    else:
                        for bb in range(2):
                            src_ap = x[bb, 256:320].rearrange(
                                "c h w -> c (h w)")
                            eng = nc.sync if bb == 0 else nc.scalar
                            eng.dma_start(out=xt[64 * bb:64 * bb + 64, :],
                                          in_=src_ap)

            def pools_for(batch):
                for i, (b, k) in enumerate(loads):
                    if b == batch:
                        pool(xts[i], z0[:, k, b, :])
                    elif b is None and batch == 0:
                        pool(xts[i], z0c2)
                        nc.gpsimd.dma_start(out=z0[0:64, 2, 0, :],
                                            in_=z0c2[0:64, :])
                        nc.gpsimd.dma_start(out=z0[0:64, 2, 1, :],
                                            in_=z0c2[64:128, :])

            # ---------------- middle: one batch-stacked chain ----------------
            with nc.named_scope("middle"):
                # GN1 + conv1 per batch: batch 0's section overlaps
                # batch 1's input DMA (engine streams execute in order)
                pzw = psA.tile([40, N], F32, tag="ps", name="pzw")
                nc.vector.memset(pzw, 0.0)
                for b in range(B):
                    pools_for(b)
                    for k in range(3):
                        st6 = ptmp.tile([128, 6], F32, tag="st6")
                        nc.vector.bn_stats(out=st6, in_=z0[:, k, b, :])
                        nc.vector.bn_aggr(out=stat2[:, k, b, :], in_=st6)
                        tm = ptmp.tile([128, 1], F32, tag="tm")
                        nc.vector.tensor_mul(tm, stat2[:, k, b, 0:1],
                                             stat2[:, k, b, 0:1])
                        nc.vector.tensor_add(stat2[:, k, b, 1:2],
                                             stat2[:, k, b, 1:2], tm)
                    pg = psA.tile([32, 2], F32, tag="ps", name=f"pg{b}")
                    for k in range(3):
                        nc.tensor.matmul(pg, gsumt[:, k, :], stat2[:, k, b, :],
                                         start=(k == 0), stop=(k == 2))
                    gm = ptmp.tile([32, 2], F32, tag="gm")
                    nc.vector.tensor_copy(gm, pg)
                    gv = ptmp.tile([32, 1], F32, tag="gv")
                    nc.vector.tensor_mul(gv, gm[:, 0:1], gm[:, 0:1])
                    nc.vector.tensor_sub(gv, gm[:, 1:2], gv)
                    nc.scalar.activation(out=gv, in_=gv, func=AF.Sqrt,
                                         bias=eps1)
                    nc.vector.reciprocal(out=ab[:, 2 * b:2 * b + 1], in_=gv)
                    nc.vector.tensor_copy(ab[:, 2 * b + 1:2 * b + 2],
                                          gm[:, 0:1])
                    pbc = psA.tile([128, 3, 2], F32, tag="ps", name=f"pbc{b}")
                    for k in range(3):
                        nc.tensor.matmul(pbc[:, k, :], gtt[:, k, :],
                                         ab[:, 2 * b:2 * b + 2],
                                         start=True, stop=True)
                    g1b = vcht[:, :, 0].unsqueeze(2)
                    b1b = vcht[:, :, 1].unsqueeze(2)
                    nc.vector.tensor_mul(
                        sbias[:, :, b, 0:1], pbc[:, :, 0:1], g1b)
                    tm2 = ptmp.tile([128, 3, 1], F32, tag="tm2")
                    nc.vector.tensor_mul(tm2, sbias[:, :, b, 0:1],
                                         pbc[:, :, 1:2])
                    nc.vector.tensor_sub(sbias[:, :, b, 1:2], b1b, tm2)
                    for k in range(3):
                        nc.scalar.activation(
                            out=pad1[:, k, b, 1:17, 1:17],
                            in_=z0[:, k, b, :].rearrange("p (h w) -> p h w",
                                                         w=16),
                            func=AF.Silu,
                            scale=sbias[:, k, b, 0:1],
                            bias=sbias[:, k, b, 1:2])
                    first = True
                    for k in range(3):
                        for ky in range(3):
                            for kx in range(3):
                                nc.tensor.matmul(
                                    pzw[32 * b:32 * b + 8, :],
                                    w1t[:, k, 3 * ky + kx, :],
                                    pad1[:, k, b, ky:ky + 16, kx:kx + 16],
                                    start=first,
                                    stop=(k == 2 and ky == 2 and kx == 2))
                                first = False
                nc.vector.tensor_scalar_add(out=zcw, in0=pzw,
                                            scalar1=c40("b_conv1"))

                # ---- attention (wide layout) ----
                st6a = ptmp.tile([40, 6], F32, tag="st6a")
                nc.vector.bn_stats(out=st6a, in_=zcw)
                mva = ptmp.tile([40, 2], F32, tag="mva")
                nc.vector.bn_aggr(out=mva, in_=st6a)
                ra = ptmp.tile([40, 1], F32, tag="ra")
                nc.scalar.activation(out=ra, in_=mva[:, 1:2], func=AF.Sqrt,
                                     bias=eps40)
                nc.vector.reciprocal(out=ra, in_=ra)
                sca = ptmp.tile([40, 2], F32, tag="sca")
                nc.vector.tensor_mul(sca[:, 0:1], ra, c40("ga"))
                tm3 = ptmp.tile([40, 1], F32, tag="tm3")
                nc.vector.tensor_mul(tm3, sca[:, 0:1], mva[:, 0:1])
                nc.vector.tensor_scalar(
                    out=sca[:, 1:2], in0=tm3, scalar1=c40("ba"), scalar2=-1.0,
                    op0=ALU.subtract, op1=ALU.mult)
                nc.vector.tensor_scalar(
                    out=znaw, in0=zcw, scalar1=sca[:, 0:1], scalar2=sca[:, 1:2],
                    op0=ALU.mult, op1=ALU.add)
                for wname, bname, dst in [("wqBD", "bq8", qsbw),
                                          ("wkBD", "bk8", ksbw),
                                          ("wvBD", "bv", vsbw)]:
                    pqkv = psA.tile([40, N], F32, tag="ps",
                                    name=f"pqkv_{wname}")
                    nc.tensor.matmul(pqkv, c40b(wname), znaw,
                                     start=True, stop=True)
                    nc.vector.tensor_scalar_add(out=dst, in0=pqkv,
                                                scalar1=c40(bname))
                qmt = c40b("qmaskW").rearrange("p (a c) -> p a c", a=2)
                nc.vector.tensor_mul(
                    qblkw,
                    qsbw.unsqueeze(1).unsqueeze(1).broadcast_to([40, 2, 2, N]),
                    qmt.unsqueeze(3).broadcast_to([40, 2, 2, N]))
                for b in range(B):
                    for mc in range(2):
                        pvt = psT.tile([128, 2, N], F32, tag="pt")
                        nc.tensor.matmul(
                            pvt[:, 0, 0:8],
                            vsbw[32 * b:32 * b + 8, 128 * mc:128 * (mc + 1)],
                            c40b("identW")[32 * b:32 * b + 8, :],
                            start=True, stop=True)
                        nc.vector.tensor_copy(vT[:, mc, b, :], pvt[:, 0, 0:8])
                wpH_t = c40b("wpHW").rearrange("p (a c) -> p a c", a=4)
                ppw = psA.tile([40, N], F32, tag="ps")
                nc.vector.memset(ppw, 0.0)
                for blk in range(2):
                    psum_s = psAcc.tile([40, 2, N], F32, tag="acc",
                                        name=f"psum_s{blk}")
                    nc.vector.memset(psum_s, 1.0)
                    psum_e = psAcc.tile([40, 2, N], F32, tag="acc",
                                        name=f"psum_e{blk}")
                    nc.vector.memset(psum_e, 0.0)
                    for b in range(B):
                        for mc in range(2):
                            pst = psT.tile([128, 2, N], F32, tag="pt")
                            nc.tensor.matmul(
                                pst.rearrange("p a n -> p (a n)"),
                                ksbw[32 * b:32 * b + 8,
                                     128 * mc:128 * (mc + 1)],
                                qblkw[32 * b:32 * b + 8, blk].rearrange(
                                    "p a n -> p (a n)"),
                                start=True, stop=True)
                            et = etp.tile([128, 2, N], BF16, tag="et")
                            nc.scalar.activation(out=et, in_=pst, func=AF.Exp)
                            etf = et.rearrange("p a n -> p (a n)")
                            nc.tensor.matmul(
                                psum_s[32 * b:32 * b + 8, :, :].rearrange(
                                    "p a n -> p (a n)"),
                                ones8x, etf, start=(mc == 0), stop=(mc == 1))
                            nc.tensor.matmul(
                                psum_e[32 * b:32 * b + 8, :, :].rearrange(
                                    "p a n -> p (a n)"),
                                vT[:, mc, b, :], etf,
                                start=(mc == 0), stop=(mc == 1))
                    den = ptmp.tile([40, 2, N], F32, tag="den")
                    nc.vector.reciprocal_approx_fast(out=den, in_=psum_s)
                    aoblk = ptmp.tile([40, 2, N], BF16, tag="aoblk")
                    nc.vector.tensor_mul(aoblk, psum_e, den)
                    # cross-head lanes: finite garbage x zero proj weight
                    for b in range(B):
                        for hp in range(2):
                            nc.tensor.matmul(
                                ppw[32 * b:32 * b + 8, :],
                                wpH_t[32 * b:32 * b + 8, 2 * blk + hp, :],
                                aoblk[32 * b:32 * b + 8, hp, :],
                                start=(blk == 0 and hp == 0),
                                stop=(blk == 1 and hp == 1))
                nc.vector.tensor_scalar_add(out=z1w, in0=ppw,
                                            scalar1=c40("b_proj"))
                nc.vector.tensor_add(z1w, z1w, zcw)

                # ---- per-pixel LN + MLP (wide) ----
                z1b = ptmp.tile([40, N], BF16, tag="z1b")
                nc.vector.tensor_copy(z1b, z1w)
                sq8 = ptmp.tile([40, N], BF16, tag="sq8")
                nc.vector.tensor_mul(sq8, z1b, z1b)
                plnA = psA.tile([2, N], F32, tag="ps", name="plnA")
                nc.tensor.matmul(plnA, c40b("w8BD"), z1b, start=True, stop=True)
                plnB = psA.tile([2, N], F32, tag="ps", name="plnB")
                nc.tensor.matmul(plnB, c40b("w8BD"), sq8, start=True, stop=True)
                muF = ptmp.tile([2, N], F32, tag="muF")
                nc.vector.tensor_copy(muF, plnA)
                muS = ptmp.tile([2, N], BF16, tag="muS")
                nc.vector.tensor_copy(muS, muF)
                musq = ptmp.tile([2, N], F32, tag="musq")
                nc.vector.tensor_mul(musq, muF, muF)
                rsS = ptmp.tile([2, N], F32, tag="rsS")
                nc.vector.tensor_sub(rsS, plnB, musq)
                nc.scalar.activation(out=rsS, in_=rsS, func=AF.Sqrt, bias=eps2p)
                rsF = ptmp.tile([2, N], F32, tag="rsF")
                nc.vector.reciprocal_approx_fast(out=rsF, in_=rsS)
                rsB = ptmp.tile([2, N], BF16, tag="rsB")
                nc.vector.tensor_copy(rsB, rsF)
                pbrM = psA.tile([40, N], F32, tag="ps", name="pbrM")
                nc.tensor.matmul(pbrM, sel2wt, muS, start=True, stop=True)
                pbrR = psA.tile([40, N], F32, tag="ps", name="pbrR")
                nc.tensor.matmul(pbrR, sel2wt, rsB, start=True, stop=True)
                cen = ptmp.tile([40, N], F32, tag="cen")
                nc.vector.tensor_sub(cen, z1w, pbrM)
                nc.vector.tensor_mul(cen, cen, pbrR)
                lnt = ptmp.tile([40, N], BF16, tag="lnt")
                nc.vector.tensor_scalar(
                    out=lnt, in0=cen, scalar1=c40("ln_g"), scalar2=c40("ln_b"),
                    op0=ALU.mult, op1=ALU.add)
                pf1 = psA.tile([48, N], F32, tag="ps")
                nc.tensor.matmul(pf1, c40b("wf1BD"), lnt, start=True, stop=True)
                hmid = ptmp.tile([48, N], BF16, tag="hmid")
                nc.scalar.activation(out=hmid, in_=pf1, func=AF.Gelu, bias=bf1wt)
                pf2 = psA.tile([40, N], F32, tag="ps")
                nc.tensor.matmul(pf2, wf2bdt, hmid, start=True, stop=True)
                nc.vector.tensor_scalar_add(out=z2w, in0=pf2,
                                            scalar1=c40("b_fc2"))
                nc.vector.tensor_add(z2w, z2w, z1w)

                # ---- GN2 + SiLU into padded tile (wide) ----
                st6b = ptmp.tile([40, 6], F32, tag="st6b")
                nc.vector.bn_stats(out=st6b, in_=z2w)
                mvb = ptmp.tile([40, 2], F32, tag="mvb")
                nc.vector.bn_aggr(out=mvb, in_=st6b)
                rb2 = ptmp.tile([40, 1], F32, tag="rb2")
                nc.scalar.activation(out=rb2, in_=mvb[:, 1:2], func=AF.Sqrt,
                                     bias=eps40)
                nc.vector.reciprocal(out=rb2, in_=rb2)
                scb = ptmp.tile([40, 2], F32, tag="scb")
                nc.vector.tensor_mul(scb[:, 0:1], rb2, c40("g2"))
                tm4 = ptmp.tile([40, 1], F32, tag="tm4")
                nc.vector.tensor_mul(tm4, scb[:, 0:1], mvb[:, 0:1])
                nc.vector.tensor_scalar(
                    out=scb[:, 1:2], in0=tm4, scalar1=c40("b2"), scalar2=-1.0,
                    op0=ALU.subtract, op1=ALU.mult)
                nc.gpsimd.memset(spadw, 0.0)
                nc.scalar.activation(
                    out=spadw[:, 1:17, 1:17],
                    in_=z2w.rearrange("p (h w) -> p h w", w=16),
                    func=AF.Silu, scale=scb[:, 0:1], bias=scb[:, 1:2])
                for dy in range(3):
                    for dx in range(3):
                        slot = 3 * dy + dx
                        for b in range(B):
                            eng = nc.sync if (slot + b) % 2 == 0 else nc.scalar
                            eng.dma_start(
                                out=sp9[8 * slot:8 * slot + 8, b, :],
                                in_=spadw[32 * b:32 * b + 8,
                                          dy:dy + 16, dx:dx + 16])

            # ---------------- conv2 + interleave + out ----------------
            sp9f = sp9.rearrange("p a b -> p (a b)")
            with nc.named_scope("conv2"):
                for c in range(3):
                    if c < 2:
                        ots = [outp.tile([128, 64, 64], F32, tag="oc",
                                         name=f"oc_b{bb}c{c}")
                               for bb in range(B)]
                    else:
                        shared = outp.tile([128, 64, 64], F32, tag="oc",
                                           name="oc_c2")
                        ots = [shared, shared]
                    for g, (pi, pj) in enumerate(GROUPS):
                        r0, nr = ROWSETS[pi]
                        c0, ncc = ROWSETS[pj]
                        pcv = psC.tile([128, B, 16, 16], F32, tag="pcv")
                        nc.tensor.matmul(
                            pcv.rearrange("p a b c -> p (a b c)"),
                            w9t[:, g, c, :], sp9f, start=True, stop=True)
                        ncopy = 0
                        for b in range(B):
                            if c < 2:
                                p0, pn = 0, 128
                            else:
                                p0, pn = 64 * b, 64
                            src_b = pcv[p0:p0 + pn, b].unsqueeze(3).broadcast_to(
                                [pn, 16, 16, ncc])
                            base5 = ots[b].rearrange(
                                "p (bi ri) (bj rj) -> p bi ri bj rj",
                                ri=4, rj=4)
                            bias_ap = vcht[p0:p0 + pn, c, 2:3]
                            for rr in range(nr):
                                dst = base5[p0:p0 + pn, :, r0 + rr, :,
                                            c0:c0 + ncc]
                                if ncopy % 2 == 0:
                                    nc.vector.tensor_scalar_add(
                                        out=dst, in0=src_b, scalar1=bias_ap)
                                else:
                                    nc.scalar.activation(
                                        out=dst, in_=src_b, func=AF.Identity,
                                        bias=bias_ap)
                                ncopy += 1
                    for b in range(B):
                        if c < 2:
                            dstd = out[b, 128 * c:128 * (c + 1)].rearrange(
                                "c h w -> c (h w)")
                            st = ots[b].rearrange("p h w -> p (h w)")
                            nc.sync.dma_start(out=dstd[:, 0:2048],
                                              in_=st[:, 0:2048])
                            nc.scalar.dma_start(out=dstd[:, 2048:4096],
                                                in_=st[:, 2048:4096])
                        else:
                            p0 = 64 * b
                            dstd = out[b, 256:320].rearrange("c h w -> c (h w)")
                            st = shared.rearrange("p h w -> p (h w)")
                            nc.sync.dma_start(out=dstd[:, 0:2048],
                                              in_=st[p0:p0 + 64, 0:2048])
                            nc.scalar.dma_start(out=dstd[:, 2048:4096],
                                                in_=st[p0:p0 + 64, 2048:4096])
    nc.compile()
    return nc


_cache = {}


def kernel(**inputs):
    x = np.ascontiguousarray(np.asarray(inputs["x"], np.float32))
    params = {k: np.asarray(v, np.float32) for k, v in inputs.items()
              if k != "x"}

    key = hash(tuple(sorted((k, v.tobytes()) for k, v in params.items())))
    if key not in _cache:
        _cache[key] = build(params)
    nc = _cache[key]

    in_maps = [{"x": np.ascontiguousarray(x[B * i:B * (i + 1)])}
               for i in range(NCORES)]
    res = run_bass_kernel_spmd(nc, in_maps, core_ids=list(range(NCORES)),
                               trace=KERNEL_TRACE)
    out = np.concatenate([res.results[i]["out"] for i in range(NCORES)], axis=0)
    if KERNEL_TRACE:
        kernel.last_result = res
    return out


# revision 29
# speedup vs baseline: 1.0421x; 1.0371x over previous
"""Trainium2 Bass kernel for the nn_Adaptor problem.

Computation (per batch image):
  avgpool4x4 -> GN(32 groups)+SiLU -> conv3x3 320->8 -> attention(4 heads) ->
  per-pixel LN + MLP -> GN(8)+SiLU -> upsample x4 nearest -> conv3x3 8->320

Distribution: pure data parallel over batch. 16 images / 8 cores = 2 per core.
Params are baked into the NEFF as inline consts (recomputed from the numpy
arrays passed to kernel() at trace time).

Implementation notes:
  - pooling keeps raw 4x4 sums (16x scale); GN1 uses eps_eff = 256*eps so the
    normalized output is exact.
  - GN1 group stats via per-channel bn_stats + grouping-matrix matmuls on PE.
  - All norm+SiLU applications fused into single scalar-engine activations.
  - conv1 as 9 shifted-window matmuls over a zero-padded 18x18 tile; both
    local batch images stacked along the matmul free dim (N=512).
  - attention: transposed scores E^T = exp(k^T q) without max subtraction
    (|scores| < 0.5 for this operator family); two heads per matmul via
    zero-masked q blocks; softmax denominators via ones-matmul column sums;
    head gather folded into zero-masked projection matmuls.
  - GN2 applied pre-upsample (nearest-upsample preserves per-channel stats).
  - conv2-after-upsample collapses to 9 phase groups with collapsed weights on
    the 16x16 grid (K=72 over a 9-slot shifted-window stack); phase outputs are
    interleaved on-chip by strided copies with step-0 column duplication, then
    written out with fully contiguous DMAs split across both HWDGE rings.
  - the middle is one batch-stacked dependency chain; engine streams execute
    in order, so fewer/wider ops beat two interleaved per-batch chains.
"""

import ml_dtypes
import numpy as np

import concourse.bass as bass
import concourse.bacc as bacc
import concourse.tile as tile
from concourse import mybir
from concourse.bass_utils import run_bass_kernel_spmd

F32 = mybir.dt.float32
BF16 = mybir.dt.bfloat16
NPBF = ml_dtypes.bfloat16
AF = mybir.ActivationFunctionType
ALU = mybir.AluOpType
AX = mybir.AxisListType

CH, C, D, HEADS = 320, 4, 8, 4
EPS = 1e-5
B = 2
N = 256
NCORES = 8

TAPSETS = {
    0: [(0, (0,)), (1, (1, 2))],
    1: [(1, (0, 1, 2))],
    3: [(1, (0, 1)), (2, (2,))],
}
GROUPS = [(pi, pj) for pi in (0, 1, 3) for pj in (0, 1, 3)]
ROWSETS = {0: (0, 1), 1: (1, 2), 3: (3, 1)}   # (start row, duplication count)

KERNEL_TRACE = False


def _conv2_tables(w_conv2):
    """W9 [72, 9, 3, 128]: collapsed per-phase-group weights over the 9-slot
    shifted-window stack; chunk 2 duplicated into cols 64:128 (two-batch
    chunk-2 matmul keeps batch 1 at psum partitions 64:128)."""
    W9 = np.zeros((72, 9, 3, 128), np.float32)
    for g, (pi, pj) in enumerate(GROUPS):
        for (dy, kys) in TAPSETS[pi]:
            for (dx, kxs) in TAPSETS[pj]:
                s = 3 * dy + dx
                wsum = np.zeros((CH, D), np.float32)
                for ky in kys:
                    for kx in kxs:
                        wsum += w_conv2[:, :, ky, kx]
                for c in range(3):
                    oc0 = 128 * c
                    ocn = min(128, CH - oc0)
                    blk = wsum[oc0:oc0 + ocn].T
                    W9[8 * s:8 * s + 8, g, c, :ocn] += blk
                    if c == 2:
                        W9[8 * s:8 * s + 8, g, c, 64:64 + ocn] += blk
    return W9


def build(params):
    P = params
    nc = bacc.Bacc("TRN2")

    x = nc.dram_tensor("x", [B, CH, 64, 64], F32, kind="ExternalInput")
    out = nc.dram_tensor("out", [B, CH, 64, 64], F32, kind="ExternalOutput")

    # ---------------- host-side constant prep ----------------
    s = float(1 / np.sqrt(D // HEADS))
    wq = P["w_qkv"].copy()
    bq = P["b_qkv"].copy()
    wq[D:2 * D] *= s
    bq[D:2 * D] *= s

    W1 = np.zeros((128, 3, 9, 8), np.float32)
    for c in range(3):
        c0 = 128 * c
        cn = min(128, CH - c0)
        for ky in range(3):
            for kx in range(3):
                W1[:cn, c, 3 * ky + kx, :] = P["w_conv1"][:, c0:c0 + cn, ky, kx].T

    W9 = _conv2_tables(P["w_conv2"])

    Gsum = np.zeros((128, 3, 32), np.float32)
    GT = np.zeros((32, 3, 128), np.float32)
    for c in range(CH):
        k, p = divmod(c, 128)
        Gsum[p, k, c // 10] = 0.1
        GT[c // 10, k, p] = 1.0

    def chunks(v, dup2=False):
        a = np.zeros((128, 3), np.float32)
        for c in range(3):
            c0 = 128 * c
            cn = min(128, CH - c0)
            a[:cn, c] = v[c0:c0 + cn]
            if dup2 and c == 2:
                a[64:64 + cn, c] = v[c0:c0 + cn]
        return a

    vch = np.stack([chunks(P["g1"]), chunks(P["b1"]),
                    chunks(P["b_conv2"], dup2=True)], axis=2)  # [128, 3, 3]

    cols8, pk8 = {}, []

    def pack8(name, arr):
        arr = np.asarray(arr, np.float32).reshape(8, -1)
        cols8[name] = (sum(a.shape[1] for a in pk8), arr.shape[1])
        pk8.append(arr)

    pack8("ones8", np.full((8, 1), 0.125, np.float32))
    for nm, val in [("bq8", bq[0:D]), ("bk8", bq[D:2 * D]), ("bv", bq[2 * D:]),
                    ("b_conv1", P["b_conv1"]), ("b_proj", P["b_proj"]),
                    ("ln_g", P["ln_g"]), ("ln_b", P["ln_b"]),
                    ("b_fc2", P["b_fc2"]), ("ga", P["ga"]), ("ba", P["ba"]),
                    ("g2", P["g2"]), ("b2", P["b2"])]:
        pack8(nm, val.reshape(8, 1))
    PK8 = np.concatenate(pk8, axis=1)

    # wide-middle consts: batch b lives at partition base 32*b
    def widen(v8, n=40):
        a = np.zeros((n, 1), np.float32)
        a[0:8, 0] = v8
        a[32:40, 0] = v8
        return a

    def blockdiag(w, n_in=40, n_out=40):
        a = np.zeros((n_in, n_out), np.float32)
        r, c = w.shape
        a[0:r, 0:c] = w
        a[32:32 + r, 32:32 + c] = w
        return a

    cols40, pk40 = {}, []

    def pack40(name, arr):
        arr = np.asarray(arr, np.float32).reshape(40, -1)
        cols40[name] = (sum(a.shape[1] for a in pk40), arr.shape[1])
        pk40.append(arr)

    for nm, val in [("bq8", bq[0:D]), ("bk8", bq[D:2 * D]), ("bv", bq[2 * D:]),
                    ("b_conv1", P["b_conv1"]), ("b_proj", P["b_proj"]),
                    ("ln_g", P["ln_g"]), ("ln_b", P["ln_b"]),
                    ("b_fc2", P["b_fc2"]), ("ga", P["ga"]), ("ba", P["ba"]),
                    ("g2", P["g2"]), ("b2", P["b2"])]:
        pack40(nm, widen(val.reshape(8)))
    PK40 = np.concatenate(pk40, axis=1)

    cols40b, pk40b = {}, []

    def pack40b(name, arr):
        arr = np.asarray(arr, np.float32)
        arr = arr.reshape(40, -1)
        cols40b[name] = (sum(a.shape[1] for a in pk40b), arr.shape[1])
        pk40b.append(arr)

    pack40b("wqBD", blockdiag(wq[0:D].T))
    pack40b("wkBD", blockdiag(wq[D:2 * D].T))
    pack40b("wvBD", blockdiag(P["w_qkv"][2 * D:3 * D].T))
    identW = np.zeros((40, 8), np.float32)
    identW[0:8] = np.eye(8)
    identW[32:40] = np.eye(8)
    pack40b("identW", identW)
    qmaskW = np.zeros((40, 4), np.float32)
    for c in range(8):
        qmaskW[c, c // 2] = 1.0
        qmaskW[32 + c, c // 2] = 1.0
    pack40b("qmaskW", qmaskW)
    wpHW = np.zeros((40, 4, 8), np.float32)
    for c in range(8):
        wpHW[c, c // 2, :] = P["w_proj"][:, c]
        wpHW[32 + c, c // 2, :] = P["w_proj"][:, c]
    pack40b("wpHW", wpHW.reshape(40, 32))
    pack40b("wf1BD", blockdiag(P["w_fc1"].T, 40, 48))
    w8BD = np.zeros((40, 2), np.float32)
    w8BD[0:8, 0] = 0.125
    w8BD[32:40, 1] = 0.125
    pack40b("w8BD", w8BD)
    PK40B = np.concatenate(pk40b, axis=1).astype(NPBF)

    WF2BD = blockdiag(P["w_fc2"].T, 48, 40).astype(NPBF)   # [48, 40]
    BF1W = np.zeros((48, 1), np.float32)
    BF1W[0:16, 0] = P["b_fc1"]
    BF1W[32:48, 0] = P["b_fc1"]
    SEL2W = np.zeros((2, 40), np.float32)
    SEL2W[0, 0:8] = 1.0
    SEL2W[1, 32:40] = 1.0
    SEL2W = SEL2W.astype(NPBF)

    cols8b, pk8b = {}, []

    def pack8b(name, arr):
        arr = np.asarray(arr, np.float32).reshape(8, -1)
        cols8b[name] = (sum(a.shape[1] for a in pk8b), arr.shape[1])
        pk8b.append(arr)

    pack8b("wqT", wq[0:D].T)
    pack8b("wkT", wq[D:2 * D].T)
    pack8b("wvT", P["w_qkv"][2 * D:3 * D].T)
    pack8b("ident8", np.eye(8, dtype=np.float32))
    qmask = np.zeros((8, 4), np.float32)
    for c in range(8):
        qmask[c, c // 2] = 1.0
    pack8b("qmask", qmask)
    wpH = np.zeros((8, 4, 8), np.float32)
    for c in range(8):
        wpH[c, c // 2, :] = P["w_proj"][:, c]
    pack8b("wpH", wpH.reshape(8, 32))
    pack8b("wf1T", P["w_fc1"].T)
    PK8B = np.concatenate(pk8b, axis=1).astype(NPBF)

    PK16 = np.concatenate([P["w_fc2"].T, P["b_fc1"].reshape(16, 1)], axis=1)
    WF2B = P["w_fc2"].T.astype(NPBF)

    h_w1 = nc.inline_tensor(W1.astype(NPBF), "cW1")
    h_w9 = nc.inline_tensor(W9.astype(NPBF), "cW9")
    h_gsum = nc.inline_tensor(Gsum, "cGsum")
    h_gt = nc.inline_tensor(GT, "cGT")
    h_vch = nc.inline_tensor(vch, "cVch")
    h_pk8 = nc.inline_tensor(PK8, "cPK8")
    h_pk40 = nc.inline_tensor(PK40, "cPK40")
    h_pk40b = nc.inline_tensor(PK40B, "cPK40B")
    h_wf2bd = nc.inline_tensor(WF2BD, "cWF2BD")
    h_bf1w = nc.inline_tensor(BF1W, "cBF1W")
    h_sel2w = nc.inline_tensor(SEL2W, "cSEL2W")
    h_pk8b = nc.inline_tensor(PK8B, "cPK8B")
    h_pk16 = nc.inline_tensor(PK16, "cPK16")
    h_wf2b = nc.inline_tensor(WF2B, "cWF2B")
    h_one18 = nc.inline_tensor(np.ones((1, 8), np.float32), "cOne18")
    h_ones128 = nc.inline_tensor(
        np.ones((128, 8), np.float32).astype(NPBF), "cOnes128")

    with tile.TileContext(nc) as tc:
        with (
            tc.tile_pool(name="consts", bufs=1) as csts,
            tc.tile_pool(name="xin", bufs=3) as xin,
            tc.tile_pool(name="pooltmp", bufs=2) as ptmp,
            tc.tile_pool(name="mid", bufs=1) as mid,
            tc.tile_pool(name="et", bufs=4) as etp,
            tc.tile_pool(name="outp", bufs=4) as outp,
            tc.tile_pool(name="psA", bufs=2, space="PSUM") as psA,
            tc.tile_pool(name="psT", bufs=1, space="PSUM") as psT,
            tc.tile_pool(name="psAcc", bufs=2, space="PSUM") as psAcc,
            tc.tile_pool(name="psC", bufs=3, space="PSUM") as psC,
        ):
            # ---------------- consts ----------------
            # order matters: gpsimd emits these serially while input DMAs
            # saturate the queues; GN1's tables go first, conv2's W9 last
            gsumt = csts.tile([128, 3, 32], F32)
            nc.gpsimd.dma_start(out=gsumt, in_=h_gsum[:])
            gtt = csts.tile([32, 3, 128], F32)
            nc.gpsimd.dma_start(out=gtt, in_=h_gt[:])
            vcht = csts.tile([128, 3, 3], F32)
            nc.gpsimd.dma_start(out=vcht, in_=h_vch[:])
            pk8t = csts.tile([8, PK8.shape[1]], F32)
            nc.gpsimd.dma_start(out=pk8t, in_=h_pk8[:])
            pk40t = csts.tile([40, PK40.shape[1]], F32)
            nc.gpsimd.dma_start(out=pk40t, in_=h_pk40[:])
            pk40bt = csts.tile([40, PK40B.shape[1]], BF16)
            nc.gpsimd.dma_start(out=pk40bt, in_=h_pk40b[:])
            w1t = csts.tile([128, 3, 9, 8], BF16)
            nc.gpsimd.dma_start(out=w1t, in_=h_w1[:])
            pk16t = csts.tile([16, 9], F32)
            nc.gpsimd.dma_start(out=pk16t, in_=h_pk16[:])
            wf2bdt = csts.tile([48, 40], BF16)
            nc.gpsimd.dma_start(out=wf2bdt, in_=h_wf2bd[:])
            bf1wt = csts.tile([48, 1], F32)
            nc.gpsimd.dma_start(out=bf1wt, in_=h_bf1w[:])
            sel2wt = csts.tile([2, 40], BF16)
            nc.gpsimd.dma_start(out=sel2wt, in_=h_sel2w[:])
            one18t = csts.tile([1, 8], F32)
            nc.gpsimd.dma_start(out=one18t, in_=h_one18[:])
            ones8x = csts.tile([128, 8], BF16)
            nc.gpsimd.dma_start(out=ones8x, in_=h_ones128[:])
            wf2bt = csts.tile([16, 8], BF16)
            nc.gpsimd.dma_start(out=wf2bt, in_=h_wf2b[:])
            pk8bt = csts.tile([8, PK8B.shape[1]], BF16)
            nc.gpsimd.dma_start(out=pk8bt, in_=h_pk8b[:])
            w9t = csts.tile([72, 9, 3, 128], BF16)
            nc.gpsimd.dma_start(out=w9t, in_=h_w9[:])

            def c8(name):
                c0, w = cols8[name]
                return pk8t[:, c0:c0 + w]

            def c40(name):
                c0, w = cols40[name]
                return pk40t[:, c0:c0 + w]

            def c40b(name):
                c0, w = cols40b[name]
                return pk40bt[:, c0:c0 + w]

            def c8b(name):
                c0, w = cols8b[name]
                return pk8bt[:, c0:c0 + w]

            bf1 = pk16t[:, 8:9]

            eps1 = csts.tile([32, 1], F32)
            nc.vector.memset(eps1, 256.0 * EPS)
            eps40 = csts.tile([40, 1], F32)
            nc.vector.memset(eps40, EPS)
            eps2p = csts.tile([2, 1], F32)
            nc.vector.memset(eps2p, EPS)

            # ---------------- state ----------------
            z0 = mid.tile([128, 3, B, N], F32)
            nc.vector.memset(z0[64:128, 2, :, :], 0.0)
            stat2 = mid.tile([128, 3, B, 2], F32)
            ab = mid.tile([32, 2 * B], F32)
            sbias = mid.tile([128, 3, B, 2], F32)
            zcw = mid.tile([40, N], F32)
            znaw = mid.tile([40, N], BF16)
            qsbw = mid.tile([40, N], BF16)
            ksbw = mid.tile([40, N], BF16)
            vsbw = mid.tile([40, N], BF16)
            qblkw = mid.tile([40, 2, 2, N], BF16)
            vT = mid.tile([128, 2, B, 8], BF16)
            z1w = mid.tile([40, N], F32)
            z2w = mid.tile([40, N], F32)
            pad1 = mid.tile([128, 3, B, 18, 18], BF16)
            spadw = mid.tile([40, 18, 18], BF16)
            sp9 = mid.tile([72, B, N], BF16)

            # ---------------- phase 1: load + pool ----------------
            def pool(xt, dst):
                wp = ptmp.tile([128, 1024], F32, tag="wp")
                for hh in range(2):
                    nc.vector.reduce_sum(
                        out=wp[:, 512 * hh:512 * hh + 512],
                        in_=xt[:, 2048 * hh:2048 * hh + 2048].rearrange(
                            "p (a b) -> p a b", b=4),
                        axis=AX.X)
                nc.vector.reduce_sum(
                    out=dst,
                    in_=wp.rearrange("p (hb hi wb) -> p hb wb hi", hi=4, wb=16),
                    axis=AX.X)

            nc.gpsimd.memset(pad1, 0.0)
            # issue every input DMA up front (sequencer streams carry only
            # DMAs, so batch-1 transfers start while batch-0 computes);
            # pool reduces are emitted per batch AFTER that batch's GN1
            # consumers are traced, keeping the Vector stream unblocked.
            loads = [(0, 0), (0, 1), (None, 2), (1, 0), (1, 1)]
            z0c2 = ptmp.tile([128, N], F32, tag="z0c2")
            xts = []
            with nc.named_scope("pool"):
                for i, (b, k) in enumerate(loads):
                    xt = xin.tile([128, 4096], F32, tag="xt", name=f"xt{i}")
                    xts.append(xt)
                    if b is not None:
                        src_ap = x[b, 128 * k:128 * (k + 1)].rearrange(
                            "c h w -> c (h w)")
                        nc.sync.dma_start(out=xt[:, 0:2048],
                                          in_=src_ap[:, 0:2048])
                        nc.scalar.dma_start(out=xt[:, 2048:4096],
                                            in_=src_ap[:, 2048:4096])
                    else:
                        for bb in range(2):
                            src_ap = x[bb, 256:320].rearrange(
                                "c h w -> c (h w)")
                            eng = nc.sync if bb == 0 else nc.scalar
                            eng.dma_start(out=xt[64 * bb:64 * bb + 64, :],
                                          in_=src_ap)

            def pools_for(batch):
                for i, (b, k) in enumerate(loads):
                    if b == batch:
                        pool(xts[i], z0[:, k, b, :])
                    elif b is None and batch == 0:
                        pool(xts[i], z0c2)
                        nc.sync.dma_start(out=z0[0:64, 2, 0, :],
                                          in_=z0c2[0:64, :])
                        nc.scalar.dma_start(out=z0[0:64, 2, 1, :],
                                            in_=z0c2[64:128, :])

            # ---------------- middle: one batch-stacked chain ----------------
            with nc.named_scope("middle"):
                # GN1 + conv1 per batch: batch 0's section overlaps
                # batch 1's input DMA (engine streams execute in order)
                pzw = psA.tile([40, N], F32, tag="ps", name="pzw")
                nc.vector.memset(pzw, 0.0)
                for b in range(B):
                    pools_for(b)
                    for k in range(3):
                        st6 = ptmp.tile([128, 6], F32, tag="st6")
                        nc.vector.bn_stats(out=st6, in_=z0[:, k, b, :])
                        nc.vector.bn_aggr(out=stat2[:, k, b, :], in_=st6)
                        tm = ptmp.tile([128, 1], F32, tag="tm")
                        nc.vector.tensor_mul(tm, stat2[:, k, b, 0:1],
                                             stat2[:, k, b, 0:1])
                        nc.vector.tensor_add(stat2[:, k, b, 1:2],
                                             stat2[:, k, b, 1:2], tm)
                    pg = psA.tile([32, 2], F32, tag="ps", name=f"pg{b}")
                    for k in range(3):
                        nc.tensor.matmul(pg, gsumt[:, k, :], stat2[:, k, b, :],
                                         start=(k == 0), stop=(k == 2))
                    gm = ptmp.tile([32, 2], F32, tag="gm")
                    nc.vector.tensor_copy(gm, pg)
                    gv = ptmp.tile([32, 1], F32, tag="gv")
                    nc.vector.tensor_mul(gv, gm[:, 0:1], gm[:, 0:1])
                    nc.vector.tensor_sub(gv, gm[:, 1:2], gv)
                    nc.scalar.activation(out=gv, in_=gv, func=AF.Sqrt,
                                         bias=eps1)
                    nc.vector.reciprocal(out=ab[:, 2 * b:2 * b + 1], in_=gv)
                    nc.vector.tensor_copy(ab[:, 2 * b + 1:2 * b + 2],
                                          gm[:, 0:1])
                    pbc = psA.tile([128, 3, 2], F32, tag="ps", name=f"pbc{b}")
                    for k in range(3):
                        nc.tensor.matmul(pbc[:, k, :], gtt[:, k, :],
                                         ab[:, 2 * b:2 * b + 2],
                                         start=True, stop=True)
                    g1b = vcht[:, :, 0].unsqueeze(2)
                    b1b = vcht[:, :, 1].unsqueeze(2)
                    nc.vector.tensor_mul(
                        sbias[:, :, b, 0:1], pbc[:, :, 0:1], g1b)
                    tm2 = ptmp.tile([128, 3, 1], F32, tag="tm2")
                    nc.vector.tensor_mul(tm2, sbias[:, :, b, 0:1],
                                         pbc[:, :, 1:2])
                    nc.vector.tensor_sub(sbias[:, :, b, 1:2], b1b, tm2)
                    for k in range(3):
                        nc.scalar.activation(
                            out=pad1[:, k, b, 1:17, 1:17],
                            in_=z0[:, k, b, :].rearrange("p (h w) -> p h w",
                                                         w=16),
                            func=AF.Silu,
                            scale=sbias[:, k, b, 0:1],
                            bias=sbias[:, k, b, 1:2])
                    first = True
                    for k in range(3):
                        for ky in range(3):
                            for kx in range(3):
                                nc.tensor.matmul(
                                    pzw[32 * b:32 * b + 8, :],
                                    w1t[:, k, 3 * ky + kx, :],
                                    pad1[:, k, b, ky:ky + 16, kx:kx + 16],
                                    start=first,
                                    stop=(k == 2 and ky == 2 and kx == 2))
                                first = False
                nc.vector.tensor_scalar_add(out=zcw, in0=pzw,
                                            scalar1=c40("b_conv1"))

                # ---- attention (wide layout) ----
                st6a = ptmp.tile([40, 6], F32, tag="st6a")
                nc.vector.bn_stats(out=st6a, in_=zcw)
                mva = ptmp.tile([40, 2], F32, tag="mva")
                nc.vector.bn_aggr(out=mva, in_=st6a)
                ra = ptmp.tile([40, 1], F32, tag="ra")
                nc.scalar.activation(out=ra, in_=mva[:, 1:2], func=AF.Sqrt,
                                     bias=eps40)
                nc.vector.reciprocal(out=ra, in_=ra)
                sca = ptmp.tile([40, 2], F32, tag="sca")
                nc.vector.tensor_mul(sca[:, 0:1], ra, c40("ga"))
                tm3 = ptmp.tile([40, 1], F32, tag="tm3")
                nc.vector.tensor_mul(tm3, sca[:, 0:1], mva[:, 0:1])
                nc.vector.tensor_scalar(
                    out=sca[:, 1:2], in0=tm3, scalar1=c40("ba"), scalar2=-1.0,
                    op0=ALU.subtract, op1=ALU.mult)
                nc.vector.tensor_scalar(
                    out=znaw, in0=zcw, scalar1=sca[:, 0:1], scalar2=sca[:, 1:2],
                    op0=ALU.mult, op1=ALU.add)
                for wname, bname, dst in [("wqBD", "bq8", qsbw),
                                          ("wkBD", "bk8", ksbw),
                                          ("wvBD", "bv", vsbw)]:
                    pqkv = psA.tile([40, N], F32, tag="ps",
                                    name=f"pqkv_{wname}")
                    nc.tensor.matmul(pqkv, c40b(wname), znaw,
                                     start=True, stop=True)
                    nc.vector.tensor_scalar_add(out=dst, in0=pqkv,
                                                scalar1=c40(bname))
                qmt = c40b("qmaskW").rearrange("p (a c) -> p a c", a=2)
                nc.vector.tensor_mul(
                    qblkw,
                    qsbw.unsqueeze(1).unsqueeze(1).broadcast_to([40, 2, 2, N]),
                    qmt.unsqueeze(3).broadcast_to([40, 2, 2, N]))
                for b in range(B):
                    for mc in range(2):
                        pvt = psT.tile([128, 2, N], F32, tag="pt")
                        nc.tensor.matmul(
                            pvt[:, 0, 0:8],
                            vsbw[32 * b:32 * b + 8, 128 * mc:128 * (mc + 1)],
                            c40b("identW")[32 * b:32 * b + 8, :],
                            start=True, stop=True)
                        nc.vector.tensor_copy(vT[:, mc, b, :], pvt[:, 0, 0:8])
                wpH_t = c40b("wpHW").rearrange("p (a c) -> p a c", a=4)
                ppw = psA.tile([40, N], F32, tag="ps")
                nc.vector.memset(ppw, 0.0)
                for blk in range(2):
                    psum_s = psAcc.tile([40, 2, N], F32, tag="acc",
                                        name=f"psum_s{blk}")
                    nc.vector.memset(psum_s, 1.0)
                    psum_e = psAcc.tile([40, 2, N], F32, tag="acc",
                                        name=f"psum_e{blk}")
                    nc.vector.memset(psum_e, 0.0)
                    for b in range(B):
                        for mc in range(2):
                            pst = psT.tile([128, 2, N], F32, tag="pt")
                            nc.tensor.matmul(
                                pst.rearrange("p a n -> p (a n)"),
                                ksbw[32 * b:32 * b + 8,
                                     128 * mc:128 * (mc + 1)],
                                qblkw[32 * b:32 * b + 8, blk].rearrange(
                                    "p a n -> p (a n)"),
                                start=True, stop=True)
                            et = etp.tile([128, 2, N], BF16, tag="et")
                            nc.scalar.activation(out=et, in_=pst, func=AF.Exp)
                            etf = et.rearrange("p a n -> p (a n)")
                            nc.tensor.matmul(
                                psum_s[32 * b:32 * b + 8, :, :].rearrange(
                                    "p a n -> p (a n)"),
                                ones8x, etf, start=(mc == 0), stop=(mc == 1))
                            nc.tensor.matmul(
                                psum_e[32 * b:32 * b + 8, :, :].rearrange(
                                    "p a n -> p (a n)"),
                                vT[:, mc, b, :], etf,
                                start=(mc == 0), stop=(mc == 1))
                    den = ptmp.tile([40, 2, N], F32, tag="den")
                    nc.vector.reciprocal_approx_fast(out=den, in_=psum_s)
                    aoblk = ptmp.tile([40, 2, N], BF16, tag="aoblk")
                    nc.vector.tensor_mul(aoblk, psum_e, den)
                    # cross-head lanes: finite garbage x zero proj weight
                    for b in range(B):
                        for hp in range(2):
                            nc.tensor.matmul(
                                ppw[32 * b:32 * b + 8, :],
                                wpH_t[32 * b:32 * b + 8, 2 * blk + hp, :],
                                aoblk[32 * b:32 * b + 8, hp, :],
                                start=(blk == 0 and hp == 0),
                                stop=(blk == 1 and hp == 1))
                nc.vector.tensor_scalar_add(out=z1w, in0=ppw,
                                            scalar1=c40("b_proj"))
                nc.vector.tensor_add(z1w, z1w, zcw)

                # ---- per-pixel LN + MLP (wide) ----
                z1b = ptmp.tile([40, N], BF16, tag="z1b")
                nc.vector.tensor_copy(z1b, z1w)
                sq8 = ptmp.tile([40, N], BF16, tag="sq8")
                nc.vector.tensor_mul(sq8, z1b, z1b)
                plnA = psA.tile([2, N], F32, tag="ps", name="plnA")
                nc.tensor.matmul(plnA, c40b("w8BD"), z1b, start=True, stop=True)
                plnB = psA.tile([2, N], F32, tag="ps", name="plnB")
                nc.tensor.matmul(plnB, c40b("w8BD"), sq8, start=True, stop=True)
                muF = ptmp.tile([2, N], F32, tag="muF")
                nc.vector.tensor_copy(muF, plnA)
                muS = ptmp.tile([2, N], BF16, tag="muS")
                nc.vector.tensor_copy(muS, muF)
                musq = ptmp.tile([2, N], F32, tag="musq")
                nc.vector.tensor_mul(musq, muF, muF)
                rsS = ptmp.tile([2, N], F32, tag="rsS")
                nc.vector.tensor_sub(rsS, plnB, musq)
                nc.scalar.activation(out=rsS, in_=rsS, func=AF.Sqrt, bias=eps2p)
                rsF = ptmp.tile([2, N], F32, tag="rsF")
                nc.vector.reciprocal_approx_fast(out=rsF, in_=rsS)
                rsB = ptmp.tile([2, N], BF16, tag="rsB")
                nc.vector.tensor_copy(rsB, rsF)
                pbrM = psA.tile([40, N], F32, tag="ps", name="pbrM")
                nc.tensor.matmul(pbrM, sel2wt, muS, start=True, stop=True)
                pbrR = psA.tile([40, N], F32, tag="ps", name="pbrR")
                nc.tensor.matmul(pbrR, sel2wt, rsB, start=True, stop=True)
                cen = ptmp.tile([40, N], F32, tag="cen")
                nc.vector.tensor_sub(cen, z1w, pbrM)
                nc.vector.tensor_mul(cen, cen, pbrR)
                lnt = ptmp.tile([40, N], BF16, tag="lnt")
                nc.vector.tensor_scalar(
                    out=lnt, in0=cen, scalar1=c40("ln_g"), scalar2=c40("ln_b"),
                    op0=ALU.mult, op1=ALU.add)
                pf1 = psA.tile([48, N], F32, tag="ps")
                nc.tensor.matmul(pf1, c40b("wf1BD"), lnt, start=True, stop=True)
                hmid = ptmp.tile([48, N], BF16, tag="hmid")
                nc.scalar.activation(out=hmid, in_=pf1, func=AF.Gelu, bias=bf1wt)
                pf2 = psA.tile([40, N], F32, tag="ps")
                nc.tensor.matmul(pf2, wf2bdt, hmid, start=True, stop=True)
                nc.vector.tensor_scalar_add(out=z2w, in0=pf2,
                                            scalar1=c40("b_fc2"))
                nc.vector.tensor_add(z2w, z2w, z1w)

                # ---- GN2 + SiLU into padded tile (wide) ----
                st6b = ptmp.tile([40, 6], F32, tag="st6b")
                nc.vector.bn_stats(out=st6b, in_=z2w)
                mvb = ptmp.tile([40, 2], F32, tag="mvb")
                nc.vector.bn_aggr(out=mvb, in_=st6b)
                rb2 = ptmp.tile([40, 1], F32, tag="rb2")
                nc.scalar.activation(out=rb2, in_=mvb[:, 1:2], func=AF.Sqrt,
                                     bias=eps40)
                nc.vector.reciprocal(out=rb2, in_=rb2)
                scb = ptmp.tile([40, 2], F32, tag="scb")
                nc.vector.tensor_mul(scb[:, 0:1], rb2, c40("g2"))
                tm4 = ptmp.tile([40, 1], F32, tag="tm4")
                nc.vector.tensor_mul(tm4, scb[:, 0:1], mvb[:, 0:1])
                nc.vector.tensor_scalar(
                    out=scb[:, 1:2], in0=tm4, scalar1=c40("b2"), scalar2=-1.0,
                    op0=ALU.subtract, op1=ALU.mult)
                nc.gpsimd.memset(spadw, 0.0)
                nc.scalar.activation(
                    out=spadw[:, 1:17, 1:17],
                    in_=z2w.rearrange("p (h w) -> p h w", w=16),
                    func=AF.Silu, scale=scb[:, 0:1], bias=scb[:, 1:2])
                for dy in range(3):
                    for dx in range(3):
                        slot = 3 * dy + dx
                        for b in range(B):
                            eng = nc.sync if (slot + b) % 2 == 0 else nc.scalar
                            eng.dma_start(
                                out=sp9[8 * slot:8 * slot + 8, b, :],
                                in_=spadw[32 * b:32 * b + 8,
                                          dy:dy + 16, dx:dx + 16])

            # ---------------- conv2 + interleave + out ----------------
            sp9f = sp9.rearrange("p a b -> p (a b)")
            with nc.named_scope("conv2"):
                for c in range(3):
                    if c < 2:
                        ots = [outp.tile([128, 64, 64], F32, tag="oc",
                                         name=f"oc_b{bb}c{c}")
                               for bb in range(B)]
                    else:
                        shared = outp.tile([128, 64, 64], F32, tag="oc",
                                           name="oc_c2")
                        ots = [shared, shared]
                    for g, (pi, pj) in enumerate(GROUPS):
                        r0, nr = ROWSETS[pi]
                        c0, ncc = ROWSETS[pj]
                        pcv = psC.tile([128, B, 16, 16], F32, tag="pcv")
                        nc.tensor.matmul(
                            pcv.rearrange("p a b c -> p (a b c)"),
                            w9t[:, g, c, :], sp9f, start=True, stop=True)
                        ncopy = 0
                        for b in range(B):
                            if c < 2:
                                p0, pn = 0, 128
                            else:
                                p0, pn = 64 * b, 64
                            src_b = pcv[p0:p0 + pn, b].unsqueeze(3).broadcast_to(
                                [pn, 16, 16, ncc])
                            base5 = ots[b].rearrange(
                                "p (bi ri) (bj rj) -> p bi ri bj rj",
                                ri=4, rj=4)
                            bias_ap = vcht[p0:p0 + pn, c, 2:3]
                            for rr in range(nr):
                                dst = base5[p0:p0 + pn, :, r0 + rr, :,
                                            c0:c0 + ncc]
                                if ncopy % 2 == 0:
                                    nc.vector.tensor_scalar_add(
                                        out=dst, in0=src_b, scalar1=bias_ap)
                                else:
                                    nc.scalar.activation(
                                        out=dst, in_=src_b, func=AF.Identity,
                                        bias=bias_ap)
                                ncopy += 1
                    for b in range(B):
                        if c < 2:
                            dstd = out[b, 128 * c:128 * (c + 1)].rearrange(
                                "c h w -> c (h w)")
                            st = ots[b].rearrange("p h w -> p (h w)")
                            nc.sync.dma_start(out=dstd[:, 0:2048],
                                              in_=st[:, 0:2048])
                            nc.scalar.dma_start(out=dstd[:, 2048:4096],
                                                in_=st[:, 2048:4096])
                        else:
                            p0 = 64 * b
                            dstd = out[b, 256:320].rearrange("c h w -> c (h w)")
                            st = shared.rearrange("p h w -> p (h w)")
                            nc.sync.dma_start(out=dstd[:, 0:2048],
                                              in_=st[p0:p0 + 64, 0:2048])
                            nc.scalar.dma_start(out=dstd[:, 2048:4096],
                                                in_=st[p0:p0 + 64, 2048:4096])
    nc.compile()
    return nc


_cache = {}


def kernel(**inputs):
    x = np.ascontiguousarray(np.asarray(inputs["x"], np.float32))
    params = {k: np.asarray(v, np.float32) for k, v in inputs.items()
              if k != "x"}

    key = hash(tuple(sorted((k, v.tobytes()) for k, v in params.items())))
    if key not in _cache:
        _cache[key] = build(params)
    nc = _cache[key]

    in_maps = [{"x": np.ascontiguousarray(x[B * i:B * (i + 1)])}
               for i in range(NCORES)]
    res = run_bass_kernel_spmd(nc, in_maps, core_ids=list(range(NCORES)),
                               trace=KERNEL_TRACE)
    out = np.concatenate([res.results[i]["out"] for i in range(NCORES)], axis=0)
    if KERNEL_TRACE:
        kernel.last_result = res
    return out


# revision 30
# speedup vs baseline: 1.0441x; 1.0019x over previous
"""Trainium2 Bass kernel for the nn_Adaptor problem.

Computation (per batch image):
  avgpool4x4 -> GN(32 groups)+SiLU -> conv3x3 320->8 -> attention(4 heads) ->
  per-pixel LN + MLP -> GN(8)+SiLU -> upsample x4 nearest -> conv3x3 8->320

Distribution: pure data parallel over batch. 16 images / 8 cores = 2 per core.
Params are baked into the NEFF as inline consts (recomputed from the numpy
arrays passed to kernel() at trace time).

Implementation notes:
  - pooling keeps raw 4x4 sums (16x scale); GN1 uses eps_eff = 256*eps so the
    normalized output is exact.
  - GN1 group stats via per-channel bn_stats + grouping-matrix matmuls on PE.
  - All norm+SiLU applications fused into single scalar-engine activations.
  - conv1 as 9 shifted-window matmuls over a zero-padded 18x18 tile; both
    local batch images stacked along the matmul free dim (N=512).
  - attention: transposed scores E^T = exp(k^T q) without max subtraction
    (|scores| < 0.5 for this operator family); two heads per matmul via
    zero-masked q blocks; softmax denominators via ones-matmul column sums;
    head gather folded into zero-masked projection matmuls.
  - GN2 applied pre-upsample (nearest-upsample preserves per-channel stats).
  - conv2-after-upsample collapses to 9 phase groups with collapsed weights on
    the 16x16 grid (K=72 over a 9-slot shifted-window stack); phase outputs are
    interleaved on-chip by strided copies with step-0 column duplication, then
    written out with fully contiguous DMAs split across both HWDGE rings.
  - the middle is one batch-stacked dependency chain; engine streams execute
    in order, so fewer/wider ops beat two interleaved per-batch chains.
"""

import ml_dtypes
import numpy as np

import concourse.bass as bass
import concourse.bacc as bacc
import concourse.tile as tile
from concourse import mybir
from concourse.bass_utils import run_bass_kernel_spmd

F32 = mybir.dt.float32
BF16 = mybir.dt.bfloat16
NPBF = ml_dtypes.bfloat16
AF = mybir.ActivationFunctionType
ALU = mybir.AluOpType
AX = mybir.AxisListType

CH, C, D, HEADS = 320, 4, 8, 4
EPS = 1e-5
B = 2
N = 256
NCORES = 8

TAPSETS = {
    0: [(0, (0,)), (1, (1, 2))],
    1: [(1, (0, 1, 2))],
    3: [(1, (0, 1)), (2, (2,))],
}
GROUPS = [(pi, pj) for pi in (0, 1, 3) for pj in (0, 1, 3)]
ROWSETS = {0: (0, 1), 1: (1, 2), 3: (3, 1)}   # (start row, duplication count)

KERNEL_TRACE = False


def _conv2_tables(w_conv2):
    """W9 [72, 9, 3, 128]: collapsed per-phase-group weights over the 9-slot
    shifted-window stack; chunk 2 duplicated into cols 64:128 (two-batch
    chunk-2 matmul keeps batch 1 at psum partitions 64:128)."""
    W9 = np.zeros((72, 9, 3, 128), np.float32)
    for g, (pi, pj) in enumerate(GROUPS):
        for (dy, kys) in TAPSETS[pi]:
            for (dx, kxs) in TAPSETS[pj]:
                s = 3 * dy + dx
                wsum = np.zeros((CH, D), np.float32)
                for ky in kys:
                    for kx in kxs:
                        wsum += w_conv2[:, :, ky, kx]
                for c in range(3):
                    oc0 = 128 * c
                    ocn = min(128, CH - oc0)
                    blk = wsum[oc0:oc0 + ocn].T
                    W9[8 * s:8 * s + 8, g, c, :ocn] += blk
                    if c == 2:
                        W9[8 * s:8 * s + 8, g, c, 64:64 + ocn] += blk
    return W9


def build(params):
    P = params
    nc = bacc.Bacc("TRN2")

    x = nc.dram_tensor("x", [B, CH, 64, 64], F32, kind="ExternalInput")
    out = nc.dram_tensor("out", [B, CH, 64, 64], F32, kind="ExternalOutput")

    # ---------------- host-side constant prep ----------------
    s = float(1 / np.sqrt(D // HEADS))
    wq = P["w_qkv"].copy()
    bq = P["b_qkv"].copy()
    wq[D:2 * D] *= s
    bq[D:2 * D] *= s

    W1 = np.zeros((128, 3, 9, 8), np.float32)
    for c in range(3):
        c0 = 128 * c
        cn = min(128, CH - c0)
        for ky in range(3):
            for kx in range(3):
                W1[:cn, c, 3 * ky + kx, :] = P["w_conv1"][:, c0:c0 + cn, ky, kx].T

    W9 = _conv2_tables(P["w_conv2"])

    Gsum = np.zeros((128, 3, 32), np.float32)
    GT = np.zeros((32, 3, 128), np.float32)
    for c in range(CH):
        k, p = divmod(c, 128)
        Gsum[p, k, c // 10] = 0.1
        GT[c // 10, k, p] = 1.0

    def chunks(v, dup2=False):
        a = np.zeros((128, 3), np.float32)
        for c in range(3):
            c0 = 128 * c
            cn = min(128, CH - c0)
            a[:cn, c] = v[c0:c0 + cn]
            if dup2 and c == 2:
                a[64:64 + cn, c] = v[c0:c0 + cn]
        return a

    vch = np.stack([chunks(P["g1"]), chunks(P["b1"]),
                    chunks(P["b_conv2"], dup2=True)], axis=2)  # [128, 3, 3]

    cols8, pk8 = {}, []

    def pack8(name, arr):
        arr = np.asarray(arr, np.float32).reshape(8, -1)
        cols8[name] = (sum(a.shape[1] for a in pk8), arr.shape[1])
        pk8.append(arr)

    pack8("ones8", np.full((8, 1), 0.125, np.float32))
    for nm, val in [("bq8", bq[0:D]), ("bk8", bq[D:2 * D]), ("bv", bq[2 * D:]),
                    ("b_conv1", P["b_conv1"]), ("b_proj", P["b_proj"]),
                    ("ln_g", P["ln_g"]), ("ln_b", P["ln_b"]),
                    ("b_fc2", P["b_fc2"]), ("ga", P["ga"]), ("ba", P["ba"]),
                    ("g2", P["g2"]), ("b2", P["b2"])]:
        pack8(nm, val.reshape(8, 1))
    PK8 = np.concatenate(pk8, axis=1)

    # wide-middle consts: batch b lives at partition base 32*b
    def widen(v8, n=40):
        a = np.zeros((n, 1), np.float32)
        a[0:8, 0] = v8
        a[32:40, 0] = v8
        return a

    def blockdiag(w, n_in=40, n_out=40):
        a = np.zeros((n_in, n_out), np.float32)
        r, c = w.shape
        a[0:r, 0:c] = w
        a[32:32 + r, 32:32 + c] = w
        return a

    cols40, pk40 = {}, []

    def pack40(name, arr):
        arr = np.asarray(arr, np.float32).reshape(40, -1)
        cols40[name] = (sum(a.shape[1] for a in pk40), arr.shape[1])
        pk40.append(arr)

    for nm, val in [("bq8", bq[0:D]), ("bk8", bq[D:2 * D]), ("bv", bq[2 * D:]),
                    ("b_conv1", P["b_conv1"]), ("b_proj", P["b_proj"]),
                    ("ln_g", P["ln_g"]), ("ln_b", P["ln_b"]),
                    ("b_fc2", P["b_fc2"]), ("ga", P["ga"]), ("ba", P["ba"]),
                    ("g2", P["g2"]), ("b2", P["b2"])]:
        pack40(nm, widen(val.reshape(8)))
    PK40 = np.concatenate(pk40, axis=1)

    cols40b, pk40b = {}, []

    def pack40b(name, arr):
        arr = np.asarray(arr, np.float32)
        arr = arr.reshape(40, -1)
        cols40b[name] = (sum(a.shape[1] for a in pk40b), arr.shape[1])
        pk40b.append(arr)

    pack40b("wqBD", blockdiag(wq[0:D].T))
    pack40b("wkBD", blockdiag(wq[D:2 * D].T))
    pack40b("wvBD", blockdiag(P["w_qkv"][2 * D:3 * D].T))
    identW = np.zeros((40, 8), np.float32)
    identW[0:8] = np.eye(8)
    identW[32:40] = np.eye(8)
    pack40b("identW", identW)
    qmaskW = np.zeros((40, 4), np.float32)
    for c in range(8):
        qmaskW[c, c // 2] = 1.0
        qmaskW[32 + c, c // 2] = 1.0
    pack40b("qmaskW", qmaskW)
    wpHW = np.zeros((40, 4, 8), np.float32)
    for c in range(8):
        wpHW[c, c // 2, :] = P["w_proj"][:, c]
        wpHW[32 + c, c // 2, :] = P["w_proj"][:, c]
    pack40b("wpHW", wpHW.reshape(40, 32))
    pack40b("wf1BD", blockdiag(P["w_fc1"].T, 40, 48))
    w8BD = np.zeros((40, 2), np.float32)
    w8BD[0:8, 0] = 0.125
    w8BD[32:40, 1] = 0.125
    pack40b("w8BD", w8BD)
    PK40B = np.concatenate(pk40b, axis=1).astype(NPBF)

    WF2BD = blockdiag(P["w_fc2"].T, 48, 40).astype(NPBF)   # [48, 40]
    BF1W = np.zeros((48, 1), np.float32)
    BF1W[0:16, 0] = P["b_fc1"]
    BF1W[32:48, 0] = P["b_fc1"]
    SEL2W = np.zeros((2, 40), np.float32)
    SEL2W[0, 0:8] = 1.0
    SEL2W[1, 32:40] = 1.0
    SEL2W = SEL2W.astype(NPBF)

    cols8b, pk8b = {}, []

    def pack8b(name, arr):
        arr = np.asarray(arr, np.float32).reshape(8, -1)
        cols8b[name] = (sum(a.shape[1] for a in pk8b), arr.shape[1])
        pk8b.append(arr)

    pack8b("wqT", wq[0:D].T)
    pack8b("wkT", wq[D:2 * D].T)
    pack8b("wvT", P["w_qkv"][2 * D:3 * D].T)
    pack8b("ident8", np.eye(8, dtype=np.float32))
    qmask = np.zeros((8, 4), np.float32)
    for c in range(8):
        qmask[c, c // 2] = 1.0
    pack8b("qmask", qmask)
    wpH = np.zeros((8, 4, 8), np.float32)
    for c in range(8):
        wpH[c, c // 2, :] = P["w_proj"][:, c]
    pack8b("wpH", wpH.reshape(8, 32))
    pack8b("wf1T", P["w_fc1"].T)
    PK8B = np.concatenate(pk8b, axis=1).astype(NPBF)

    PK16 = np.concatenate([P["w_fc2"].T, P["b_fc1"].reshape(16, 1)], axis=1)
    WF2B = P["w_fc2"].T.astype(NPBF)

    h_w1 = nc.inline_tensor(W1.astype(NPBF), "cW1")
    h_w9 = nc.inline_tensor(W9.astype(NPBF), "cW9")
    h_gsum = nc.inline_tensor(Gsum, "cGsum")
    h_gt = nc.inline_tensor(GT, "cGT")
    h_vch = nc.inline_tensor(vch, "cVch")
    h_pk8 = nc.inline_tensor(PK8, "cPK8")
    h_pk40 = nc.inline_tensor(PK40, "cPK40")
    h_pk40b = nc.inline_tensor(PK40B, "cPK40B")
    h_wf2bd = nc.inline_tensor(WF2BD, "cWF2BD")
    h_bf1w = nc.inline_tensor(BF1W, "cBF1W")
    h_sel2w = nc.inline_tensor(SEL2W, "cSEL2W")
    h_pk8b = nc.inline_tensor(PK8B, "cPK8B")
    h_pk16 = nc.inline_tensor(PK16, "cPK16")
    h_wf2b = nc.inline_tensor(WF2B, "cWF2B")
    h_one18 = nc.inline_tensor(np.ones((1, 8), np.float32), "cOne18")
    h_ones128 = nc.inline_tensor(
        np.ones((128, 8), np.float32).astype(NPBF), "cOnes128")

    with tile.TileContext(nc) as tc:
        with (
            tc.tile_pool(name="consts", bufs=1) as csts,
            tc.tile_pool(name="xin", bufs=3) as xin,
            tc.tile_pool(name="pooltmp", bufs=2) as ptmp,
            tc.tile_pool(name="mid", bufs=1) as mid,
            tc.tile_pool(name="et", bufs=4) as etp,
            tc.tile_pool(name="outp", bufs=4) as outp,
            tc.tile_pool(name="psA", bufs=2, space="PSUM") as psA,
            tc.tile_pool(name="psT", bufs=1, space="PSUM") as psT,
            tc.tile_pool(name="psAcc", bufs=2, space="PSUM") as psAcc,
            tc.tile_pool(name="psC", bufs=3, space="PSUM") as psC,
        ):
            # ---------------- consts ----------------
            # order matters: gpsimd emits these serially while input DMAs
            # saturate the queues; GN1's tables go first, conv2's W9 last
            gsumt = csts.tile([128, 3, 32], F32)
            nc.gpsimd.dma_start(out=gsumt, in_=h_gsum[:])
            gtt = csts.tile([32, 3, 128], F32)
            nc.gpsimd.dma_start(out=gtt, in_=h_gt[:])
            vcht = csts.tile([128, 3, 3], F32)
            nc.gpsimd.dma_start(out=vcht, in_=h_vch[:])
            pk8t = csts.tile([8, PK8.shape[1]], F32)
            nc.gpsimd.dma_start(out=pk8t, in_=h_pk8[:])
            pk40t = csts.tile([40, PK40.shape[1]], F32)
            nc.gpsimd.dma_start(out=pk40t, in_=h_pk40[:])
            pk40bt = csts.tile([40, PK40B.shape[1]], BF16)
            nc.gpsimd.dma_start(out=pk40bt, in_=h_pk40b[:])
            w1t = csts.tile([128, 3, 9, 8], BF16)
            nc.gpsimd.dma_start(out=w1t, in_=h_w1[:])
            pk16t = csts.tile([16, 9], F32)
            nc.gpsimd.dma_start(out=pk16t, in_=h_pk16[:])
            wf2bdt = csts.tile([48, 40], BF16)
            nc.gpsimd.dma_start(out=wf2bdt, in_=h_wf2bd[:])
            bf1wt = csts.tile([48, 1], F32)
            nc.gpsimd.dma_start(out=bf1wt, in_=h_bf1w[:])
            sel2wt = csts.tile([2, 40], BF16)
            nc.gpsimd.dma_start(out=sel2wt, in_=h_sel2w[:])
            one18t = csts.tile([1, 8], F32)
            nc.gpsimd.dma_start(out=one18t, in_=h_one18[:])
            ones8x = csts.tile([128, 8], BF16)
            nc.gpsimd.dma_start(out=ones8x, in_=h_ones128[:])
            wf2bt = csts.tile([16, 8], BF16)
            nc.gpsimd.dma_start(out=wf2bt, in_=h_wf2b[:])
            pk8bt = csts.tile([8, PK8B.shape[1]], BF16)
            nc.gpsimd.dma_start(out=pk8bt, in_=h_pk8b[:])
            w9t = csts.tile([72, 9, 3, 128], BF16)
            nc.gpsimd.dma_start(out=w9t, in_=h_w9[:])

            def c8(name):
                c0, w = cols8[name]
                return pk8t[:, c0:c0 + w]

            def c40(name):
                c0, w = cols40[name]
                return pk40t[:, c0:c0 + w]

            def c40b(name):
                c0, w = cols40b[name]
                return pk40bt[:, c0:c0 + w]

            def c8b(name):
                c0, w = cols8b[name]
                return pk8bt[:, c0:c0 + w]

            bf1 = pk16t[:, 8:9]

            eps1 = csts.tile([32, 1], F32)
            nc.vector.memset(eps1, 256.0 * EPS)
            eps40 = csts.tile([40, 1], F32)
            nc.vector.memset(eps40, EPS)
            eps2p = csts.tile([2, 1], F32)
            nc.vector.memset(eps2p, EPS)

            # ---------------- state ----------------
            z0 = mid.tile([128, 3, B, N], F32)
            nc.vector.memset(z0[64:128, 2, :, :], 0.0)
            stat2 = mid.tile([128, 3, B, 2], F32)
            ab = mid.tile([32, 2 * B], F32)
            sbias = mid.tile([128, 3, B, 2], F32)
            zcw = mid.tile([40, N], F32)
            znaw = mid.tile([40, N], BF16)
            qsbw = mid.tile([40, N], BF16)
            ksbw = mid.tile([40, N], BF16)
            vsbw = mid.tile([40, N], BF16)
            qblkw = mid.tile([40, 2, 2, N], BF16)
            vT = mid.tile([128, 2, B, 8], BF16)
            z1w = mid.tile([40, N], F32)
            z2w = mid.tile([40, N], F32)
            pad1 = mid.tile([128, 3, B, 18, 18], BF16)
            spadw = mid.tile([40, 18, 18], BF16)
            sp9 = mid.tile([72, B, N], BF16)

            # ---------------- phase 1: load + pool ----------------
            def pool(xt, dst):
                wp = ptmp.tile([128, 1024], F32, tag="wp")
                for hh in range(2):
                    nc.vector.reduce_sum(
                        out=wp[:, 512 * hh:512 * hh + 512],
                        in_=xt[:, 2048 * hh:2048 * hh + 2048].rearrange(
                            "p (a b) -> p a b", b=4),
                        axis=AX.X)
                nc.vector.reduce_sum(
                    out=dst,
                    in_=wp.rearrange("p (hb hi wb) -> p hb wb hi", hi=4, wb=16),
                    axis=AX.X)

            nc.gpsimd.memset(pad1, 0.0)
            # issue every input DMA up front (sequencer streams carry only
            # DMAs, so batch-1 transfers start while batch-0 computes);
            # pool reduces are emitted per batch AFTER that batch's GN1
            # consumers are traced, keeping the Vector stream unblocked.
            loads = [(0, 0), (0, 1), (None, 2), (1, 0), (1, 1)]
            z0c2 = ptmp.tile([128, N], F32, tag="z0c2")
            xts = []
            with nc.named_scope("pool"):
                for i, (b, k) in enumerate(loads):
                    xt = xin.tile([128, 4096], F32, tag="xt", name=f"xt{i}")
                    xts.append(xt)
                    if b is not None:
                        src_ap = x[b, 128 * k:128 * (k + 1)].rearrange(
                            "c h w -> c (h w)")
                        nc.sync.dma_start(out=xt[:, 0:2048],
                                          in_=src_ap[:, 0:2048])
                        nc.scalar.dma_start(out=xt[:, 2048:4096],
                                            in_=src_ap[:, 2048:4096])
                    else:
                        for bb in range(2):
                            src_ap = x[bb, 256:320].rearrange(
                                "c h w -> c (h w)")
                            eng = nc.sync if bb == 0 else nc.scalar
                            eng.dma_start(out=xt[64 * bb:64 * bb + 64, :],
                                          in_=src_ap)

            def pool_gp(xt, dst):
                # batch-1 pooling on GpSimd so it runs concurrently with
                # batch-0's GN1 chain on Vector (engine streams are in-order)
                wp = ptmp.tile([128, 1024], F32, tag="wpg")
                for hh in range(2):
                    xv = xt[:, 2048 * hh:2048 * hh + 2048].rearrange(
                        "p (a b) -> p a b", b=4)
                    w2 = wp[:, 512 * hh:512 * hh + 512]
                    t01 = ptmp.tile([128, 512], F32, tag="t01")
                    nc.gpsimd.tensor_add(t01, xv[:, :, 0], xv[:, :, 1])
                    t23 = ptmp.tile([128, 512], F32, tag="t23")
                    nc.gpsimd.tensor_add(t23, xv[:, :, 2], xv[:, :, 3])
                    nc.gpsimd.tensor_add(w2, t01, t23)
                wpv = wp.rearrange("p (hb hi wb) -> p hb hi wb", hi=4, wb=16)
                h01 = ptmp.tile([128, 16, 16], F32, tag="h01")
                nc.gpsimd.tensor_add(h01, wpv[:, :, 0, :], wpv[:, :, 1, :])
                h23 = ptmp.tile([128, 16, 16], F32, tag="h23")
                nc.gpsimd.tensor_add(h23, wpv[:, :, 2, :], wpv[:, :, 3, :])
                nc.gpsimd.tensor_add(dst, h01, h23)

            def pools_for(batch):
                for i, (b, k) in enumerate(loads):
                    if b == batch:
                        if batch == 1:
                            pool_gp(xts[i], z0[:, k, b, :])
                        else:
                            pool(xts[i], z0[:, k, b, :])
                    elif b is None and batch == 0:
                        pool(xts[i], z0c2)
                        nc.sync.dma_start(out=z0[0:64, 2, 0, :],
                                          in_=z0c2[0:64, :])
                        nc.scalar.dma_start(out=z0[0:64, 2, 1, :],
                                            in_=z0c2[64:128, :])

            # ---------------- middle: one batch-stacked chain ----------------
            with nc.named_scope("middle"):
                # GN1 + conv1 per batch: batch 0's section overlaps
                # batch 1's input DMA (engine streams execute in order)
                pzw = psA.tile([40, N], F32, tag="ps", name="pzw")
                nc.vector.memset(pzw, 0.0)
                for b in range(B):
                    pools_for(b)
                    for k in range(3):
                        st6 = ptmp.tile([128, 6], F32, tag="st6")
                        nc.vector.bn_stats(out=st6, in_=z0[:, k, b, :])
                        nc.vector.bn_aggr(out=stat2[:, k, b, :], in_=st6)
                        tm = ptmp.tile([128, 1], F32, tag="tm")
                        nc.vector.tensor_mul(tm, stat2[:, k, b, 0:1],
                                             stat2[:, k, b, 0:1])
                        nc.vector.tensor_add(stat2[:, k, b, 1:2],
                                             stat2[:, k, b, 1:2], tm)
                    pg = psA.tile([32, 2], F32, tag="ps", name=f"pg{b}")
                    for k in range(3):
                        nc.tensor.matmul(pg, gsumt[:, k, :], stat2[:, k, b, :],
                                         start=(k == 0), stop=(k == 2))
                    gm = ptmp.tile([32, 2], F32, tag="gm")
                    nc.vector.tensor_copy(gm, pg)
                    gv = ptmp.tile([32, 1], F32, tag="gv")
                    nc.vector.tensor_mul(gv, gm[:, 0:1], gm[:, 0:1])
                    nc.vector.tensor_sub(gv, gm[:, 1:2], gv)
                    nc.scalar.activation(out=gv, in_=gv, func=AF.Sqrt,
                                         bias=eps1)
                    nc.vector.reciprocal(out=ab[:, 2 * b:2 * b + 1], in_=gv)
                    nc.vector.tensor_copy(ab[:, 2 * b + 1:2 * b + 2],
                                          gm[:, 0:1])
                    pbc = psA.tile([128, 3, 2], F32, tag="ps", name=f"pbc{b}")
                    for k in range(3):
                        nc.tensor.matmul(pbc[:, k, :], gtt[:, k, :],
                                         ab[:, 2 * b:2 * b + 2],
                                         start=True, stop=True)
                    g1b = vcht[:, :, 0].unsqueeze(2)
                    b1b = vcht[:, :, 1].unsqueeze(2)
                    nc.vector.tensor_mul(
                        sbias[:, :, b, 0:1], pbc[:, :, 0:1], g1b)
                    tm2 = ptmp.tile([128, 3, 1], F32, tag="tm2")
                    nc.vector.tensor_mul(tm2, sbias[:, :, b, 0:1],
                                         pbc[:, :, 1:2])
                    nc.vector.tensor_sub(sbias[:, :, b, 1:2], b1b, tm2)
                    for k in range(3):
                        nc.scalar.activation(
                            out=pad1[:, k, b, 1:17, 1:17],
                            in_=z0[:, k, b, :].rearrange("p (h w) -> p h w",
                                                         w=16),
                            func=AF.Silu,
                            scale=sbias[:, k, b, 0:1],
                            bias=sbias[:, k, b, 1:2])
                    first = True
                    for k in range(3):
                        for ky in range(3):
                            for kx in range(3):
                                nc.tensor.matmul(
                                    pzw[32 * b:32 * b + 8, :],
                                    w1t[:, k, 3 * ky + kx, :],
                                    pad1[:, k, b, ky:ky + 16, kx:kx + 16],
                                    start=first,
                                    stop=(k == 2 and ky == 2 and kx == 2))
                                first = False
                nc.vector.tensor_scalar_add(out=zcw, in0=pzw,
                                            scalar1=c40("b_conv1"))

                # ---- attention (wide layout) ----
                st6a = ptmp.tile([40, 6], F32, tag="st6a")
                nc.vector.bn_stats(out=st6a, in_=zcw)
                mva = ptmp.tile([40, 2], F32, tag="mva")
                nc.vector.bn_aggr(out=mva, in_=st6a)
                ra = ptmp.tile([40, 1], F32, tag="ra")
                nc.scalar.activation(out=ra, in_=mva[:, 1:2], func=AF.Sqrt,
                                     bias=eps40)
                nc.vector.reciprocal(out=ra, in_=ra)
                sca = ptmp.tile([40, 2], F32, tag="sca")
                nc.vector.tensor_mul(sca[:, 0:1], ra, c40("ga"))
                tm3 = ptmp.tile([40, 1], F32, tag="tm3")
                nc.vector.tensor_mul(tm3, sca[:, 0:1], mva[:, 0:1])
                nc.vector.tensor_scalar(
                    out=sca[:, 1:2], in0=tm3, scalar1=c40("ba"), scalar2=-1.0,
                    op0=ALU.subtract, op1=ALU.mult)
                nc.vector.tensor_scalar(
                    out=znaw, in0=zcw, scalar1=sca[:, 0:1], scalar2=sca[:, 1:2],
                    op0=ALU.mult, op1=ALU.add)
                for wname, bname, dst in [("wqBD", "bq8", qsbw),
                                          ("wkBD", "bk8", ksbw),
                                          ("wvBD", "bv", vsbw)]:
                    pqkv = psA.tile([40, N], F32, tag="ps",
                                    name=f"pqkv_{wname}")
                    nc.tensor.matmul(pqkv, c40b(wname), znaw,
                                     start=True, stop=True)
                    nc.vector.tensor_scalar_add(out=dst, in0=pqkv,
                                                scalar1=c40(bname))
                qmt = c40b("qmaskW").rearrange("p (a c) -> p a c", a=2)
                nc.vector.tensor_mul(
                    qblkw,
                    qsbw.unsqueeze(1).unsqueeze(1).broadcast_to([40, 2, 2, N]),
                    qmt.unsqueeze(3).broadcast_to([40, 2, 2, N]))
                for b in range(B):
                    for mc in range(2):
                        pvt = psT.tile([128, 2, N], F32, tag="pt")
                        nc.tensor.matmul(
                            pvt[:, 0, 0:8],
                            vsbw[32 * b:32 * b + 8, 128 * mc:128 * (mc + 1)],
                            c40b("identW")[32 * b:32 * b + 8, :],
                            start=True, stop=True)
                        nc.vector.tensor_copy(vT[:, mc, b, :], pvt[:, 0, 0:8])
                wpH_t = c40b("wpHW").rearrange("p (a c) -> p a c", a=4)
                ppw = psA.tile([40, N], F32, tag="ps")
                nc.vector.memset(ppw, 0.0)
                for blk in range(2):
                    psum_s = psAcc.tile([40, 2, N], F32, tag="acc",
                                        name=f"psum_s{blk}")
                    nc.vector.memset(psum_s, 1.0)
                    psum_e = psAcc.tile([40, 2, N], F32, tag="acc",
                                        name=f"psum_e{blk}")
                    nc.vector.memset(psum_e, 0.0)
                    for b in range(B):
                        for mc in range(2):
                            pst = psT.tile([128, 2, N], F32, tag="pt")
                            nc.tensor.matmul(
                                pst.rearrange("p a n -> p (a n)"),
                                ksbw[32 * b:32 * b + 8,
                                     128 * mc:128 * (mc + 1)],
                                qblkw[32 * b:32 * b + 8, blk].rearrange(
                                    "p a n -> p (a n)"),
                                start=True, stop=True)
                            et = etp.tile([128, 2, N], BF16, tag="et")
                            nc.scalar.activation(out=et, in_=pst, func=AF.Exp)
                            etf = et.rearrange("p a n -> p (a n)")
                            nc.tensor.matmul(
                                psum_s[32 * b:32 * b + 8, :, :].rearrange(
                                    "p a n -> p (a n)"),
                                ones8x, etf, start=(mc == 0), stop=(mc == 1))
                            nc.tensor.matmul(
                                psum_e[32 * b:32 * b + 8, :, :].rearrange(
                                    "p a n -> p (a n)"),
                                vT[:, mc, b, :], etf,
                                start=(mc == 0), stop=(mc == 1))
                    den = ptmp.tile([40, 2, N], F32, tag="den")
                    nc.vector.reciprocal_approx_fast(out=den, in_=psum_s)
                    aoblk = ptmp.tile([40, 2, N], BF16, tag="aoblk")
                    nc.vector.tensor_mul(aoblk, psum_e, den)
                    # cross-head lanes: finite garbage x zero proj weight
                    for b in range(B):
                        for hp in range(2):
                            nc.tensor.matmul(
                                ppw[32 * b:32 * b + 8, :],
                                wpH_t[32 * b:32 * b + 8, 2 * blk + hp, :],
                                aoblk[32 * b:32 * b + 8, hp, :],
                                start=(blk == 0 and hp == 0),
                                stop=(blk == 1 and hp == 1))
                nc.vector.tensor_scalar_add(out=z1w, in0=ppw,
                                            scalar1=c40("b_proj"))
                nc.vector.tensor_add(z1w, z1w, zcw)

                # ---- per-pixel LN + MLP (wide) ----
                z1b = ptmp.tile([40, N], BF16, tag="z1b")
                nc.vector.tensor_copy(z1b, z1w)
                sq8 = ptmp.tile([40, N], BF16, tag="sq8")
                nc.vector.tensor_mul(sq8, z1b, z1b)
                plnA = psA.tile([2, N], F32, tag="ps", name="plnA")
                nc.tensor.matmul(plnA, c40b("w8BD"), z1b, start=True, stop=True)
                plnB = psA.tile([2, N], F32, tag="ps", name="plnB")
                nc.tensor.matmul(plnB, c40b("w8BD"), sq8, start=True, stop=True)
                muF = ptmp.tile([2, N], F32, tag="muF")
                nc.vector.tensor_copy(muF, plnA)
                muS = ptmp.tile([2, N], BF16, tag="muS")
                nc.vector.tensor_copy(muS, muF)
                musq = ptmp.tile([2, N], F32, tag="musq")
                nc.vector.tensor_mul(musq, muF, muF)
                rsS = ptmp.tile([2, N], F32, tag="rsS")
                nc.vector.tensor_sub(rsS, plnB, musq)
                nc.scalar.activation(out=rsS, in_=rsS, func=AF.Sqrt, bias=eps2p)
                rsF = ptmp.tile([2, N], F32, tag="rsF")
                nc.vector.reciprocal_approx_fast(out=rsF, in_=rsS)
                rsB = ptmp.tile([2, N], BF16, tag="rsB")
                nc.vector.tensor_copy(rsB, rsF)
                pbrM = psA.tile([40, N], F32, tag="ps", name="pbrM")
                nc.tensor.matmul(pbrM, sel2wt, muS, start=True, stop=True)
                pbrR = psA.tile([40, N], F32, tag="ps", name="pbrR")
                nc.tensor.matmul(pbrR, sel2wt, rsB, start=True, stop=True)
                cen = ptmp.tile([40, N], F32, tag="cen")
                nc.vector.tensor_sub(cen, z1w, pbrM)
                nc.vector.tensor_mul(cen, cen, pbrR)
                lnt = ptmp.tile([40, N], BF16, tag="lnt")
                nc.vector.tensor_scalar(
                    out=lnt, in0=cen, scalar1=c40("ln_g"), scalar2=c40("ln_b"),
                    op0=ALU.mult, op1=ALU.add)
                pf1 = psA.tile([48, N], F32, tag="ps")
                nc.tensor.matmul(pf1, c40b("wf1BD"), lnt, start=True, stop=True)
                hmid = ptmp.tile([48, N], BF16, tag="hmid")
                nc.scalar.activation(out=hmid, in_=pf1, func=AF.Gelu, bias=bf1wt)
                pf2 = psA.tile([40, N], F32, tag="ps")
                nc.tensor.matmul(pf2, wf2bdt, hmid, start=True, stop=True)
                nc.vector.tensor_scalar_add(out=z2w, in0=pf2,
                                            scalar1=c40("b_fc2"))
                nc.vector.tensor_add(z2w, z2w, z1w)

                # ---- GN2 + SiLU into padded tile (wide) ----
                st6b = ptmp.tile([40, 6], F32, tag="st6b")
                nc.vector.bn_stats(out=st6b, in_=z2w)
                mvb = ptmp.tile([40, 2], F32, tag="mvb")
                nc.vector.bn_aggr(out=mvb, in_=st6b)
                rb2 = ptmp.tile([40, 1], F32, tag="rb2")
                nc.scalar.activation(out=rb2, in_=mvb[:, 1:2], func=AF.Sqrt,
                                     bias=eps40)
                nc.vector.reciprocal(out=rb2, in_=rb2)
                scb = ptmp.tile([40, 2], F32, tag="scb")
                nc.vector.tensor_mul(scb[:, 0:1], rb2, c40("g2"))
                tm4 = ptmp.tile([40, 1], F32, tag="tm4")
                nc.vector.tensor_mul(tm4, scb[:, 0:1], mvb[:, 0:1])
                nc.vector.tensor_scalar(
                    out=scb[:, 1:2], in0=tm4, scalar1=c40("b2"), scalar2=-1.0,
                    op0=ALU.subtract, op1=ALU.mult)
                nc.gpsimd.memset(spadw, 0.0)
                nc.scalar.activation(
                    out=spadw[:, 1:17, 1:17],
                    in_=z2w.rearrange("p (h w) -> p h w", w=16),
                    func=AF.Silu, scale=scb[:, 0:1], bias=scb[:, 1:2])
                for dy in range(3):
                    for dx in range(3):
                        slot = 3 * dy + dx
                        for b in range(B):
                            eng = nc.sync if (slot + b) % 2 == 0 else nc.scalar
                            eng.dma_start(
                                out=sp9[8 * slot:8 * slot + 8, b, :],
                                in_=spadw[32 * b:32 * b + 8,
                                          dy:dy + 16, dx:dx + 16])

            # ---------------- conv2 + interleave + out ----------------
            sp9f = sp9.rearrange("p a b -> p (a b)")
            with nc.named_scope("conv2"):
                for c in range(3):
                    if c < 2:
                        ots = [outp.tile([128, 64, 64], F32, tag="oc",
                                         name=f"oc_b{bb}c{c}")
                               for bb in range(B)]
                    else:
                        shared = outp.tile([128, 64, 64], F32, tag="oc",
                                           name="oc_c2")
                        ots = [shared, shared]
                    for g, (pi, pj) in enumerate(GROUPS):
                        r0, nr = ROWSETS[pi]
                        c0, ncc = ROWSETS[pj]
                        pcv = psC.tile([128, B, 16, 16], F32, tag="pcv")
                        nc.tensor.matmul(
                            pcv.rearrange("p a b c -> p (a b c)"),
                            w9t[:, g, c, :], sp9f, start=True, stop=True)
                        ncopy = 0
                        for b in range(B):
                            if c < 2:
                                p0, pn = 0, 128
                            else:
                                p0, pn = 64 * b, 64
                            src_b = pcv[p0:p0 + pn, b].unsqueeze(3).broadcast_to(
                                [pn, 16, 16, ncc])
                            base5 = ots[b].rearrange(
                                "p (bi ri) (bj rj) -> p bi ri bj rj",
                                ri=4, rj=4)
                            bias_ap = vcht[p0:p0 + pn, c, 2:3]
                            for rr in range(nr):
                                dst = base5[p0:p0 + pn, :, r0 + rr, :,
                                            c0:c0 + ncc]
                                if ncopy % 2 == 0:
                                    nc.vector.tensor_scalar_add(
                                        out=dst, in0=src_b, scalar1=bias_ap)
                                else:
                                    nc.scalar.activation(
                                        out=dst, in_=src_b, func=AF.Identity,
                                        bias=bias_ap)
                                ncopy += 1
                    for b in range(B):
                        if c < 2:
                            dstd = out[b, 128 * c:128 * (c + 1)].rearrange(
                                "c h w -> c (h w)")
                            st = ots[b].rearrange("p h w -> p (h w)")
                            nc.sync.dma_start(out=dstd[:, 0:2048],
                                              in_=st[:, 0:2048])
                            nc.scalar.dma_start(out=dstd[:, 2048:4096],
                                                in_=st[:, 2048:4096])
                        else:
                            p0 = 64 * b
                            dstd = out[b, 256:320].rearrange("c h w -> c (h w)")
                            st = shared.rearrange("p h w -> p (h w)")
                            nc.sync.dma_start(out=dstd[:, 0:2048],
                                              in_=st[p0:p0 + 64, 0:2048])
                            nc.scalar.dma_start(out=dstd[:, 2048:4096],
                                                in_=st[p0:p0 + 64, 2048:4096])
    nc.compile()
    return nc


_cache = {}


def kernel(**inputs):
    x = np.ascontiguousarray(np.asarray(inputs["x"], np.float32))
    params = {k: np.asarray(v, np.float32) for k, v in inputs.items()
              if k != "x"}

    key = hash(tuple(sorted((k, v.tobytes()) for k, v in params.items())))
    if key not in _cache:
        _cache[key] = build(params)
    nc = _cache[key]

    in_maps = [{"x": np.ascontiguousarray(x[B * i:B * (i + 1)])}
               for i in range(NCORES)]
    res = run_bass_kernel_spmd(nc, in_maps, core_ids=list(range(NCORES)),
                               trace=KERNEL_TRACE)
    out = np.concatenate([res.results[i]["out"] for i in range(NCORES)], axis=0)
    if KERNEL_TRACE:
        kernel.last_result = res
    return out


# revision 31
# speedup vs baseline: 1.0649x; 1.0199x over previous
"""Trainium2 Bass kernel for the nn_Adaptor problem.

Computation (per batch image):
  avgpool4x4 -> GN(32 groups)+SiLU -> conv3x3 320->8 -> attention(4 heads) ->
  per-pixel LN + MLP -> GN(8)+SiLU -> upsample x4 nearest -> conv3x3 8->320

Distribution: pure data parallel over batch. 16 images / 8 cores = 2 per core.
Params are baked into the NEFF as inline consts (recomputed from the numpy
arrays passed to kernel() at trace time).

Implementation notes:
  - pooling keeps raw 4x4 sums (16x scale); GN1 uses eps_eff = 256*eps so the
    normalized output is exact.
  - GN1 group stats via per-channel bn_stats + grouping-matrix matmuls on PE.
  - All norm+SiLU applications fused into single scalar-engine activations.
  - conv1 as 9 shifted-window matmuls over a zero-padded 18x18 tile; both
    local batch images stacked along the matmul free dim (N=512).
  - attention: transposed scores E^T = exp(k^T q) without max subtraction
    (|scores| < 0.5 for this operator family); two heads per matmul via
    zero-masked q blocks; softmax denominators via ones-matmul column sums;
    head gather folded into zero-masked projection matmuls.
  - GN2 applied pre-upsample (nearest-upsample preserves per-channel stats).
  - conv2-after-upsample collapses to 9 phase groups with collapsed weights on
    the 16x16 grid (K=72 over a 9-slot shifted-window stack); phase outputs are
    interleaved on-chip by strided copies with step-0 column duplication, then
    written out with fully contiguous DMAs split across both HWDGE rings.
  - the middle is one batch-stacked dependency chain; engine streams execute
    in order, so fewer/wider ops beat two interleaved per-batch chains.
"""

import ml_dtypes
import numpy as np

import concourse.bass as bass
import concourse.bacc as bacc
import concourse.tile as tile
from concourse import mybir
from concourse.bass_utils import run_bass_kernel_spmd

F32 = mybir.dt.float32
BF16 = mybir.dt.bfloat16
NPBF = ml_dtypes.bfloat16
AF = mybir.ActivationFunctionType
ALU = mybir.AluOpType
AX = mybir.AxisListType

CH, C, D, HEADS = 320, 4, 8, 4
EPS = 1e-5
B = 2
N = 256
NCORES = 8

TAPSETS = {
    0: [(0, (0,)), (1, (1, 2))],
    1: [(1, (0, 1, 2))],
    3: [(1, (0, 1)), (2, (2,))],
}
GROUPS = [(pi, pj) for pi in (0, 1, 3) for pj in (0, 1, 3)]
ROWSETS = {0: (0, 1), 1: (1, 2), 3: (3, 1)}   # (start row, duplication count)

KERNEL_TRACE = False


def _conv2_tables(w_conv2):
    """W9 [72, 9, 3, 128]: collapsed per-phase-group weights over the 9-slot
    shifted-window stack; chunk 2 duplicated into cols 64:128 (two-batch
    chunk-2 matmul keeps batch 1 at psum partitions 64:128)."""
    W9 = np.zeros((72, 9, 3, 128), np.float32)
    for g, (pi, pj) in enumerate(GROUPS):
        for (dy, kys) in TAPSETS[pi]:
            for (dx, kxs) in TAPSETS[pj]:
                s = 3 * dy + dx
                wsum = np.zeros((CH, D), np.float32)
                for ky in kys:
                    for kx in kxs:
                        wsum += w_conv2[:, :, ky, kx]
                for c in range(3):
                    oc0 = 128 * c
                    ocn = min(128, CH - oc0)
                    blk = wsum[oc0:oc0 + ocn].T
                    W9[8 * s:8 * s + 8, g, c, :ocn] += blk
                    if c == 2:
                        W9[8 * s:8 * s + 8, g, c, 64:64 + ocn] += blk
    return W9


def build(params):
    P = params
    nc = bacc.Bacc("TRN2")

    x = nc.dram_tensor("x", [B, CH, 64, 64], F32, kind="ExternalInput")
    out = nc.dram_tensor("out", [B, CH, 64, 64], F32, kind="ExternalOutput")

    # ---------------- host-side constant prep ----------------
    s = float(1 / np.sqrt(D // HEADS))
    wq = P["w_qkv"].copy()
    bq = P["b_qkv"].copy()
    wq[D:2 * D] *= s
    bq[D:2 * D] *= s

    W1 = np.zeros((128, 3, 9, 8), np.float32)
    for c in range(3):
        c0 = 128 * c
        cn = min(128, CH - c0)
        for ky in range(3):
            for kx in range(3):
                W1[:cn, c, 3 * ky + kx, :] = P["w_conv1"][:, c0:c0 + cn, ky, kx].T

    W9 = _conv2_tables(P["w_conv2"])

    Gsum = np.zeros((128, 3, 32), np.float32)
    GT = np.zeros((32, 3, 128), np.float32)
    for c in range(CH):
        k, p = divmod(c, 128)
        Gsum[p, k, c // 10] = 0.1
        GT[c // 10, k, p] = 1.0

    def chunks(v, dup2=False):
        a = np.zeros((128, 3), np.float32)
        for c in range(3):
            c0 = 128 * c
            cn = min(128, CH - c0)
            a[:cn, c] = v[c0:c0 + cn]
            if dup2 and c == 2:
                a[64:64 + cn, c] = v[c0:c0 + cn]
        return a

    vch = np.stack([chunks(P["g1"]), chunks(P["b1"]),
                    chunks(P["b_conv2"], dup2=True)], axis=2)  # [128, 3, 3]

    cols8, pk8 = {}, []

    def pack8(name, arr):
        arr = np.asarray(arr, np.float32).reshape(8, -1)
        cols8[name] = (sum(a.shape[1] for a in pk8), arr.shape[1])
        pk8.append(arr)

    pack8("ones8", np.full((8, 1), 0.125, np.float32))
    for nm, val in [("bq8", bq[0:D]), ("bk8", bq[D:2 * D]), ("bv", bq[2 * D:]),
                    ("b_conv1", P["b_conv1"]), ("b_proj", P["b_proj"]),
                    ("ln_g", P["ln_g"]), ("ln_b", P["ln_b"]),
                    ("b_fc2", P["b_fc2"]), ("ga", P["ga"]), ("ba", P["ba"]),
                    ("g2", P["g2"]), ("b2", P["b2"])]:
        pack8(nm, val.reshape(8, 1))
    PK8 = np.concatenate(pk8, axis=1)

    # wide-middle consts: batch b lives at partition base 32*b
    def widen(v8, n=40):
        a = np.zeros((n, 1), np.float32)
        a[0:8, 0] = v8
        a[32:40, 0] = v8
        return a

    def blockdiag(w, n_in=40, n_out=40):
        a = np.zeros((n_in, n_out), np.float32)
        r, c = w.shape
        a[0:r, 0:c] = w
        a[32:32 + r, 32:32 + c] = w
        return a

    cols40, pk40 = {}, []

    def pack40(name, arr):
        arr = np.asarray(arr, np.float32).reshape(40, -1)
        cols40[name] = (sum(a.shape[1] for a in pk40), arr.shape[1])
        pk40.append(arr)

    for nm, val in [("bq8", bq[0:D]), ("bk8", bq[D:2 * D]), ("bv", bq[2 * D:]),
                    ("b_conv1", P["b_conv1"]), ("b_proj", P["b_proj"]),
                    ("ln_g", P["ln_g"]), ("ln_b", P["ln_b"]),
                    ("b_fc2", P["b_fc2"]), ("ga", P["ga"]), ("ba", P["ba"]),
                    ("g2", P["g2"]), ("b2", P["b2"])]:
        pack40(nm, widen(val.reshape(8)))
    PK40 = np.concatenate(pk40, axis=1)

    cols40b, pk40b = {}, []

    def pack40b(name, arr):
        arr = np.asarray(arr, np.float32)
        arr = arr.reshape(40, -1)
        cols40b[name] = (sum(a.shape[1] for a in pk40b), arr.shape[1])
        pk40b.append(arr)

    pack40b("wqBD", blockdiag(wq[0:D].T))
    pack40b("wkBD", blockdiag(wq[D:2 * D].T))
    pack40b("wvBD", blockdiag(P["w_qkv"][2 * D:3 * D].T))
    identW = np.zeros((40, 8), np.float32)
    identW[0:8] = np.eye(8)
    identW[32:40] = np.eye(8)
    pack40b("identW", identW)
    qmaskW = np.zeros((40, 4), np.float32)
    for c in range(8):
        qmaskW[c, c // 2] = 1.0
        qmaskW[32 + c, c // 2] = 1.0
    pack40b("qmaskW", qmaskW)
    wpHW = np.zeros((40, 4, 8), np.float32)
    for c in range(8):
        wpHW[c, c // 2, :] = P["w_proj"][:, c]
        wpHW[32 + c, c // 2, :] = P["w_proj"][:, c]
    pack40b("wpHW", wpHW.reshape(40, 32))
    pack40b("wf1BD", blockdiag(P["w_fc1"].T, 40, 48))
    w8BD = np.zeros((40, 2), np.float32)
    w8BD[0:8, 0] = 0.125
    w8BD[32:40, 1] = 0.125
    pack40b("w8BD", w8BD)
    PK40B = np.concatenate(pk40b, axis=1).astype(NPBF)

    WF2BD = blockdiag(P["w_fc2"].T, 48, 40).astype(NPBF)   # [48, 40]
    BF1W = np.zeros((48, 1), np.float32)
    BF1W[0:16, 0] = P["b_fc1"]
    BF1W[32:48, 0] = P["b_fc1"]
    SEL2W = np.zeros((2, 40), np.float32)
    SEL2W[0, 0:8] = 1.0
    SEL2W[1, 32:40] = 1.0
    SEL2W = SEL2W.astype(NPBF)

    cols8b, pk8b = {}, []

    def pack8b(name, arr):
        arr = np.asarray(arr, np.float32).reshape(8, -1)
        cols8b[name] = (sum(a.shape[1] for a in pk8b), arr.shape[1])
        pk8b.append(arr)

    pack8b("wqT", wq[0:D].T)
    pack8b("wkT", wq[D:2 * D].T)
    pack8b("wvT", P["w_qkv"][2 * D:3 * D].T)
    pack8b("ident8", np.eye(8, dtype=np.float32))
    qmask = np.zeros((8, 4), np.float32)
    for c in range(8):
        qmask[c, c // 2] = 1.0
    pack8b("qmask", qmask)
    wpH = np.zeros((8, 4, 8), np.float32)
    for c in range(8):
        wpH[c, c // 2, :] = P["w_proj"][:, c]
    pack8b("wpH", wpH.reshape(8, 32))
    pack8b("wf1T", P["w_fc1"].T)
    PK8B = np.concatenate(pk8b, axis=1).astype(NPBF)

    PK16 = np.concatenate([P["w_fc2"].T, P["b_fc1"].reshape(16, 1)], axis=1)
    WF2B = P["w_fc2"].T.astype(NPBF)

    h_w1 = nc.inline_tensor(W1.astype(NPBF), "cW1")
    h_w9 = nc.inline_tensor(W9.astype(NPBF), "cW9")
    h_gsum = nc.inline_tensor(Gsum, "cGsum")
    h_gt = nc.inline_tensor(GT, "cGT")
    h_vch = nc.inline_tensor(vch, "cVch")
    h_pk8 = nc.inline_tensor(PK8, "cPK8")
    h_pk40 = nc.inline_tensor(PK40, "cPK40")
    h_pk40b = nc.inline_tensor(PK40B, "cPK40B")
    h_wf2bd = nc.inline_tensor(WF2BD, "cWF2BD")
    h_bf1w = nc.inline_tensor(BF1W, "cBF1W")
    h_sel2w = nc.inline_tensor(SEL2W, "cSEL2W")
    h_pk8b = nc.inline_tensor(PK8B, "cPK8B")
    h_pk16 = nc.inline_tensor(PK16, "cPK16")
    h_wf2b = nc.inline_tensor(WF2B, "cWF2B")
    h_one18 = nc.inline_tensor(np.ones((1, 8), np.float32), "cOne18")
    h_ones128 = nc.inline_tensor(
        np.ones((128, 8), np.float32).astype(NPBF), "cOnes128")

    with tile.TileContext(nc) as tc:
        with (
            tc.tile_pool(name="consts", bufs=1) as csts,
            tc.tile_pool(name="xin", bufs=3) as xin,
            tc.tile_pool(name="pooltmp", bufs=2) as ptmp,
            tc.tile_pool(name="mid", bufs=1) as mid,
            tc.tile_pool(name="et", bufs=4) as etp,
            tc.tile_pool(name="outp", bufs=4) as outp,
            tc.tile_pool(name="psA", bufs=2, space="PSUM") as psA,
            tc.tile_pool(name="psT", bufs=1, space="PSUM") as psT,
            tc.tile_pool(name="psAcc", bufs=2, space="PSUM") as psAcc,
            tc.tile_pool(name="psC", bufs=3, space="PSUM") as psC,
        ):
            # ---------------- consts ----------------
            w1t = csts.tile([128, 3, 9, 8], BF16)
            nc.gpsimd.dma_start(out=w1t, in_=h_w1[:])
            w9t = csts.tile([72, 9, 3, 128], BF16)
            nc.gpsimd.dma_start(out=w9t, in_=h_w9[:])
            gsumt = csts.tile([128, 3, 32], F32)
            nc.gpsimd.dma_start(out=gsumt, in_=h_gsum[:])
            gtt = csts.tile([32, 3, 128], F32)
            nc.gpsimd.dma_start(out=gtt, in_=h_gt[:])
            vcht = csts.tile([128, 3, 3], F32)
            nc.gpsimd.dma_start(out=vcht, in_=h_vch[:])
            pk8t = csts.tile([8, PK8.shape[1]], F32)
            nc.gpsimd.dma_start(out=pk8t, in_=h_pk8[:])
            pk40t = csts.tile([40, PK40.shape[1]], F32)
            nc.gpsimd.dma_start(out=pk40t, in_=h_pk40[:])
            pk40bt = csts.tile([40, PK40B.shape[1]], BF16)
            nc.gpsimd.dma_start(out=pk40bt, in_=h_pk40b[:])
            wf2bdt = csts.tile([48, 40], BF16)
            nc.gpsimd.dma_start(out=wf2bdt, in_=h_wf2bd[:])
            bf1wt = csts.tile([48, 1], F32)
            nc.gpsimd.dma_start(out=bf1wt, in_=h_bf1w[:])
            sel2wt = csts.tile([2, 40], BF16)
            nc.gpsimd.dma_start(out=sel2wt, in_=h_sel2w[:])
            pk8bt = csts.tile([8, PK8B.shape[1]], BF16)
            nc.gpsimd.dma_start(out=pk8bt, in_=h_pk8b[:])
            pk16t = csts.tile([16, 9], F32)
            nc.gpsimd.dma_start(out=pk16t, in_=h_pk16[:])
            wf2bt = csts.tile([16, 8], BF16)
            nc.gpsimd.dma_start(out=wf2bt, in_=h_wf2b[:])
            one18t = csts.tile([1, 8], F32)
            nc.gpsimd.dma_start(out=one18t, in_=h_one18[:])
            ones8x = csts.tile([128, 8], BF16)
            nc.gpsimd.dma_start(out=ones8x, in_=h_ones128[:])

            def c8(name):
                c0, w = cols8[name]
                return pk8t[:, c0:c0 + w]

            def c40(name):
                c0, w = cols40[name]
                return pk40t[:, c0:c0 + w]

            def c40b(name):
                c0, w = cols40b[name]
                return pk40bt[:, c0:c0 + w]

            def c8b(name):
                c0, w = cols8b[name]
                return pk8bt[:, c0:c0 + w]

            bf1 = pk16t[:, 8:9]

            eps1 = csts.tile([32, 1], F32)
            nc.vector.memset(eps1, 256.0 * EPS)
            eps40 = csts.tile([40, 1], F32)
            nc.vector.memset(eps40, EPS)
            eps2p = csts.tile([2, 1], F32)
            nc.vector.memset(eps2p, EPS)

            # ---------------- state ----------------
            z0 = mid.tile([128, 3, B, N], F32)
            nc.vector.memset(z0[64:128, 2, :, :], 0.0)
            stat2 = mid.tile([128, 3, B, 2], F32)
            ab = mid.tile([32, 2 * B], F32)
            sbias = mid.tile([128, 3, B, 2], F32)
            zcw = mid.tile([40, N], F32)
            znaw = mid.tile([40, N], BF16)
            qsbw = mid.tile([40, N], BF16)
            ksbw = mid.tile([40, N], BF16)
            vsbw = mid.tile([40, N], BF16)
            qblkw = mid.tile([40, 2, 2, N], BF16)
            vT = mid.tile([128, 2, B, 8], BF16)
            z1w = mid.tile([40, N], F32)
            z2w = mid.tile([40, N], F32)
            pad1 = mid.tile([128, 3, B, 18, 18], BF16)
            spadw = mid.tile([40, 18, 18], BF16)
            sp9 = mid.tile([72, B, N], BF16)

            # ---------------- phase 1: load + pool ----------------
            def pool(xt, dst):
                wp = ptmp.tile([128, 1024], F32, tag="wp")
                for hh in range(2):
                    nc.vector.reduce_sum(
                        out=wp[:, 512 * hh:512 * hh + 512],
                        in_=xt[:, 2048 * hh:2048 * hh + 2048].rearrange(
                            "p (a b) -> p a b", b=4),
                        axis=AX.X)
                wpv = wp.rearrange("p (hb hi wb) -> p hb hi wb", hi=4, wb=16)
                t01 = ptmp.tile([128, 16, 16], F32, tag="t01")
                nc.gpsimd.tensor_add(t01, wpv[:, :, 0, :], wpv[:, :, 1, :])
                t23 = ptmp.tile([128, 16, 16], F32, tag="t23")
                nc.gpsimd.tensor_add(t23, wpv[:, :, 2, :], wpv[:, :, 3, :])
                nc.gpsimd.tensor_add(dst, t01, t23)

            loads = [(0, 0), (0, 1), (None, 2), (1, 0), (1, 1)]
            z0c2 = ptmp.tile([128, N], F32, tag="z0c2")
            with nc.named_scope("pool"):
                for b, k in loads:
                    xt = xin.tile([128, 4096], F32, tag="xt")
                    if b is not None:
                        src_ap = x[b, 128 * k:128 * (k + 1)].rearrange(
                            "c h w -> c (h w)")
                        nc.sync.dma_start(out=xt[:, 0:2048], in_=src_ap[:, 0:2048])
                        nc.scalar.dma_start(out=xt[:, 2048:4096],
                                            in_=src_ap[:, 2048:4096])
                        pool(xt, z0[:, k, b, :])
                    else:
                        for bb in range(2):
                            src_ap = x[bb, 256:320].rearrange("c h w -> c (h w)")
                            eng = nc.sync if bb == 0 else nc.scalar
                            eng.dma_start(out=xt[64 * bb:64 * bb + 64, :],
                                          in_=src_ap)
                        pool(xt, z0c2)
                        nc.gpsimd.dma_start(out=z0[0:64, 2, 0, :],
                                            in_=z0c2[0:64, :])
                        nc.gpsimd.dma_start(out=z0[0:64, 2, 1, :],
                                            in_=z0c2[64:128, :])

            # ---------------- middle: one batch-stacked chain ----------------
            with nc.named_scope("middle"):
                # GN1 per-channel stats
                for k in range(3):
                    for b in range(B):
                        st6 = ptmp.tile([128, 6], F32, tag="st6")
                        nc.vector.bn_stats(out=st6, in_=z0[:, k, b, :])
                        nc.vector.bn_aggr(out=stat2[:, k, b, :], in_=st6)
                    tm = ptmp.tile([128, 2], F32, tag="tm")
                    nc.vector.tensor_mul(tm, stat2[:, k, :, 0], stat2[:, k, :, 0])
                    nc.vector.tensor_add(stat2[:, k, :, 1], stat2[:, k, :, 1], tm)
                pg = psA.tile([32, 4], F32, tag="ps")
                for k in range(3):
                    nc.tensor.matmul(
                        pg, gsumt[:, k, :],
                        stat2[:, k, :, :].rearrange("p a b -> p (a b)"),
                        start=(k == 0), stop=(k == 2))
                gm = ptmp.tile([32, 4], F32, tag="gm")
                nc.vector.tensor_copy(gm, pg)
                gv = ptmp.tile([32, 2], F32, tag="gv")
                nc.vector.tensor_mul(gv, gm[:, 0::2], gm[:, 0::2])
                nc.vector.tensor_sub(gv, gm[:, 1::2], gv)
                nc.scalar.activation(out=gv, in_=gv, func=AF.Sqrt, bias=eps1)
                nc.vector.reciprocal(out=ab[:, 0::2], in_=gv)
                nc.vector.tensor_copy(ab[:, 1::2], gm[:, 0::2])
                for k in range(3):
                    pbc = psA.tile([128, 4], F32, tag="ps")
                    nc.tensor.matmul(pbc, gtt[:, k, :], ab,
                                     start=True, stop=True)
                    nc.vector.tensor_scalar_mul(
                        out=sbias[:, k, :, 0], in0=pbc[:, 0::2],
                        scalar1=vcht[:, k, 0:1])
                    tm2 = ptmp.tile([128, 2], F32, tag="tm2")
                    nc.vector.tensor_mul(tm2, sbias[:, k, :, 0], pbc[:, 1::2])
                    nc.vector.tensor_scalar(
                        out=sbias[:, k, :, 1], in0=tm2,
                        scalar1=vcht[:, k, 1:2], scalar2=-1.0,
                        op0=ALU.subtract, op1=ALU.mult)
                nc.gpsimd.memset(pad1, 0.0)
                for k in range(3):
                    for b in range(B):
                        nc.scalar.activation(
                            out=pad1[:, k, b, 1:17, 1:17],
                            in_=z0[:, k, b, :].rearrange("p (h w) -> p h w", w=16),
                            func=AF.Silu,
                            scale=sbias[:, k, b, 0:1], bias=sbias[:, k, b, 1:2])
                # conv1: per-batch accumulation chains into a wide psum
                # (batch b at partition base 32b; rows 8..31 stay memset-zero)
                pzw = psA.tile([40, N], F32, tag="ps")
                nc.vector.memset(pzw, 0.0)
                for b in range(B):
                    first = True
                    for k in range(3):
                        for ky in range(3):
                            for kx in range(3):
                                nc.tensor.matmul(
                                    pzw[32 * b:32 * b + 8, :],
                                    w1t[:, k, 3 * ky + kx, :],
                                    pad1[:, k, b, ky:ky + 16, kx:kx + 16],
                                    start=first,
                                    stop=(k == 2 and ky == 2 and kx == 2))
                                first = False
                nc.vector.tensor_scalar_add(out=zcw, in0=pzw,
                                            scalar1=c40("b_conv1"))

                # ---- attention (wide layout) ----
                st6a = ptmp.tile([40, 6], F32, tag="st6a")
                nc.vector.bn_stats(out=st6a, in_=zcw)
                mva = ptmp.tile([40, 2], F32, tag="mva")
                nc.vector.bn_aggr(out=mva, in_=st6a)
                ra = ptmp.tile([40, 1], F32, tag="ra")
                nc.scalar.activation(out=ra, in_=mva[:, 1:2], func=AF.Sqrt,
                                     bias=eps40)
                nc.vector.reciprocal(out=ra, in_=ra)
                sca = ptmp.tile([40, 2], F32, tag="sca")
                nc.vector.tensor_mul(sca[:, 0:1], ra, c40("ga"))
                tm3 = ptmp.tile([40, 1], F32, tag="tm3")
                nc.vector.tensor_mul(tm3, sca[:, 0:1], mva[:, 0:1])
                nc.vector.tensor_scalar(
                    out=sca[:, 1:2], in0=tm3, scalar1=c40("ba"), scalar2=-1.0,
                    op0=ALU.subtract, op1=ALU.mult)
                nc.scalar.activation(out=znaw, in_=zcw, func=AF.Identity,
                                     scale=sca[:, 0:1], bias=sca[:, 1:2])
                for wname, bname, dst in [("wqBD", "bq8", qsbw),
                                          ("wkBD", "bk8", ksbw),
                                          ("wvBD", "bv", vsbw)]:
                    pqkv = psA.tile([40, N], F32, tag="ps",
                                    name=f"pqkv_{wname}")
                    nc.tensor.matmul(pqkv, c40b(wname), znaw,
                                     start=True, stop=True)
                    nc.vector.tensor_scalar_add(out=dst, in0=pqkv,
                                                scalar1=c40(bname))
                qmt = c40b("qmaskW").rearrange("p (a c) -> p a c", a=2)
                nc.vector.tensor_mul(
                    qblkw,
                    qsbw.unsqueeze(1).unsqueeze(1).broadcast_to([40, 2, 2, N]),
                    qmt.unsqueeze(3).broadcast_to([40, 2, 2, N]))
                for b in range(B):
                    for mc in range(2):
                        pvt = psT.tile([128, 2, N], F32, tag="pt")
                        nc.tensor.matmul(
                            pvt[:, 0, 0:8],
                            vsbw[32 * b:32 * b + 8, 128 * mc:128 * (mc + 1)],
                            c40b("identW")[32 * b:32 * b + 8, :],
                            start=True, stop=True)
                        nc.vector.tensor_copy(vT[:, mc, b, :], pvt[:, 0, 0:8])
                wpH_t = c40b("wpHW").rearrange("p (a c) -> p a c", a=4)
                ppw = psA.tile([40, N], F32, tag="ps")
                nc.vector.memset(ppw, 0.0)
                for blk in range(2):
                    psum_s = psAcc.tile([40, 2, N], F32, tag="acc",
                                        name=f"psum_s{blk}")
                    nc.vector.memset(psum_s, 1.0)
                    psum_e = psAcc.tile([40, 2, N], F32, tag="acc",
                                        name=f"psum_e{blk}")
                    nc.vector.memset(psum_e, 0.0)
                    for b in range(B):
                        for mc in range(2):
                            pst = psT.tile([128, 2, N], F32, tag="pt")
                            nc.tensor.matmul(
                                pst.rearrange("p a n -> p (a n)"),
                                ksbw[32 * b:32 * b + 8,
                                     128 * mc:128 * (mc + 1)],
                                qblkw[32 * b:32 * b + 8, blk].rearrange(
                                    "p a n -> p (a n)"),
                                start=True, stop=True)
                            et = etp.tile([128, 2, N], BF16, tag="et")
                            nc.scalar.activation(out=et, in_=pst, func=AF.Exp)
                            etf = et.rearrange("p a n -> p (a n)")
                            nc.tensor.matmul(
                                psum_s[32 * b:32 * b + 8, :, :].rearrange(
                                    "p a n -> p (a n)"),
                                ones8x, etf, start=(mc == 0), stop=(mc == 1))
                            nc.tensor.matmul(
                                psum_e[32 * b:32 * b + 8, :, :].rearrange(
                                    "p a n -> p (a n)"),
                                vT[:, mc, b, :], etf,
                                start=(mc == 0), stop=(mc == 1))
                    den = ptmp.tile([40, 2, N], F32, tag="den")
                    nc.vector.reciprocal_approx_fast(out=den, in_=psum_s)
                    aoblk = ptmp.tile([40, 2, N], BF16, tag="aoblk")
                    nc.vector.tensor_mul(aoblk, psum_e, den)
                    # cross-head lanes: finite garbage x zero proj weight
                    for b in range(B):
                        for hp in range(2):
                            nc.tensor.matmul(
                                ppw[32 * b:32 * b + 8, :],
                                wpH_t[32 * b:32 * b + 8, 2 * blk + hp, :],
                                aoblk[32 * b:32 * b + 8, hp, :],
                                start=(blk == 0 and hp == 0),
                                stop=(blk == 1 and hp == 1))
                nc.vector.tensor_scalar_add(out=z1w, in0=ppw,
                                            scalar1=c40("b_proj"))
                nc.vector.tensor_add(z1w, z1w, zcw)

                # ---- per-pixel LN + MLP (wide) ----
                z1b = ptmp.tile([40, N], BF16, tag="z1b")
                nc.vector.tensor_copy(z1b, z1w)
                sq8 = ptmp.tile([40, N], BF16, tag="sq8")
                nc.vector.tensor_mul(sq8, z1b, z1b)
                plnA = psA.tile([2, N], F32, tag="ps", name="plnA")
                nc.tensor.matmul(plnA, c40b("w8BD"), z1b, start=True, stop=True)
                plnB = psA.tile([2, N], F32, tag="ps", name="plnB")
                nc.tensor.matmul(plnB, c40b("w8BD"), sq8, start=True, stop=True)
                muF = ptmp.tile([2, N], F32, tag="muF")
                nc.vector.tensor_copy(muF, plnA)
                muS = ptmp.tile([2, N], BF16, tag="muS")
                nc.vector.tensor_copy(muS, muF)
                musq = ptmp.tile([2, N], F32, tag="musq")
                nc.vector.tensor_mul(musq, muF, muF)
                rsS = ptmp.tile([2, N], F32, tag="rsS")
                nc.vector.tensor_sub(rsS, plnB, musq)
                nc.scalar.activation(out=rsS, in_=rsS, func=AF.Sqrt, bias=eps2p)
                rsF = ptmp.tile([2, N], F32, tag="rsF")
                nc.vector.reciprocal_approx_fast(out=rsF, in_=rsS)
                rsB = ptmp.tile([2, N], BF16, tag="rsB")
                nc.vector.tensor_copy(rsB, rsF)
                pbrM = psA.tile([40, N], F32, tag="ps", name="pbrM")
                nc.tensor.matmul(pbrM, sel2wt, muS, start=True, stop=True)
                pbrR = psA.tile([40, N], F32, tag="ps", name="pbrR")
                nc.tensor.matmul(pbrR, sel2wt, rsB, start=True, stop=True)
                cen = ptmp.tile([40, N], F32, tag="cen")
                nc.vector.tensor_sub(cen, z1w, pbrM)
                nc.vector.tensor_mul(cen, cen, pbrR)
                lnt = ptmp.tile([40, N], BF16, tag="lnt")
                nc.vector.tensor_scalar(
                    out=lnt, in0=cen, scalar1=c40("ln_g"), scalar2=c40("ln_b"),
                    op0=ALU.mult, op1=ALU.add)
                pf1 = psA.tile([48, N], F32, tag="ps")
                nc.tensor.matmul(pf1, c40b("wf1BD"), lnt, start=True, stop=True)
                hmid = ptmp.tile([48, N], BF16, tag="hmid")
                nc.scalar.activation(out=hmid, in_=pf1, func=AF.Gelu, bias=bf1wt)
                pf2 = psA.tile([40, N], F32, tag="ps")
                nc.tensor.matmul(pf2, wf2bdt, hmid, start=True, stop=True)
                nc.vector.tensor_scalar_add(out=z2w, in0=pf2,
                                            scalar1=c40("b_fc2"))
                nc.vector.tensor_add(z2w, z2w, z1w)

                # ---- GN2 + SiLU into padded tile (wide) ----
                st6b = ptmp.tile([40, 6], F32, tag="st6b")
                nc.vector.bn_stats(out=st6b, in_=z2w)
                mvb = ptmp.tile([40, 2], F32, tag="mvb")
                nc.vector.bn_aggr(out=mvb, in_=st6b)
                rb2 = ptmp.tile([40, 1], F32, tag="rb2")
                nc.scalar.activation(out=rb2, in_=mvb[:, 1:2], func=AF.Sqrt,
                                     bias=eps40)
                nc.vector.reciprocal(out=rb2, in_=rb2)
                scb = ptmp.tile([40, 2], F32, tag="scb")
                nc.vector.tensor_mul(scb[:, 0:1], rb2, c40("g2"))
                tm4 = ptmp.tile([40, 1], F32, tag="tm4")
                nc.vector.tensor_mul(tm4, scb[:, 0:1], mvb[:, 0:1])
                nc.vector.tensor_scalar(
                    out=scb[:, 1:2], in0=tm4, scalar1=c40("b2"), scalar2=-1.0,
                    op0=ALU.subtract, op1=ALU.mult)
                nc.gpsimd.memset(spadw, 0.0)
                nc.scalar.activation(
                    out=spadw[:, 1:17, 1:17],
                    in_=z2w.rearrange("p (h w) -> p h w", w=16),
                    func=AF.Silu, scale=scb[:, 0:1], bias=scb[:, 1:2])
                for dy in range(3):
                    for dx in range(3):
                        slot = 3 * dy + dx
                        for b in range(B):
                            eng = nc.sync if (slot + b) % 2 == 0 else nc.scalar
                            eng.dma_start(
                                out=sp9[8 * slot:8 * slot + 8, b, :],
                                in_=spadw[32 * b:32 * b + 8,
                                          dy:dy + 16, dx:dx + 16])

            # ---------------- conv2 + interleave + out ----------------
            sp9f = sp9.rearrange("p a b -> p (a b)")
            with nc.named_scope("conv2"):
                for c in range(3):
                    if c < 2:
                        ots = [outp.tile([128, 64, 64], F32, tag="oc",
                                         name=f"oc_b{bb}c{c}")
                               for bb in range(B)]
                    else:
                        shared = outp.tile([128, 64, 64], F32, tag="oc",
                                           name="oc_c2")
                        ots = [shared, shared]
                    for g, (pi, pj) in enumerate(GROUPS):
                        r0, nr = ROWSETS[pi]
                        c0, ncc = ROWSETS[pj]
                        pcv = psC.tile([128, B, 16, 16], F32, tag="pcv")
                        nc.tensor.matmul(
                            pcv.rearrange("p a b c -> p (a b c)"),
                            w9t[:, g, c, :], sp9f, start=True, stop=True)
                        ncopy = 0
                        for b in range(B):
                            if c < 2:
                                p0, pn = 0, 128
                            else:
                                p0, pn = 64 * b, 64
                            src_b = pcv[p0:p0 + pn, b].unsqueeze(3).broadcast_to(
                                [pn, 16, 16, ncc])
                            base5 = ots[b].rearrange(
                                "p (bi ri) (bj rj) -> p bi ri bj rj",
                                ri=4, rj=4)
                            bias_ap = vcht[p0:p0 + pn, c, 2:3]
                            for rr in range(nr):
                                dst = base5[p0:p0 + pn, :, r0 + rr, :,
                                            c0:c0 + ncc]
                                if ncopy % 2 == 0:
                                    nc.vector.tensor_scalar_add(
                                        out=dst, in0=src_b, scalar1=bias_ap)
                                else:
                                    nc.scalar.activation(
                                        out=dst, in_=src_b, func=AF.Identity,
                                        bias=bias_ap)
                                ncopy += 1
                    for b in range(B):
                        if c < 2:
                            dstd = out[b, 128 * c:128 * (c + 1)].rearrange(
                                "c h w -> c (h w)")
                            st = ots[b].rearrange("p h w -> p (h w)")
                            nc.sync.dma_start(out=dstd[:, 0:2048],
                                              in_=st[:, 0:2048])
                            nc.scalar.dma_start(out=dstd[:, 2048:4096],
                                                in_=st[:, 2048:4096])
                        else:
                            p0 = 64 * b
                            dstd = out[b, 256:320].rearrange("c h w -> c (h w)")
                            st = shared.rearrange("p h w -> p (h w)")
                            nc.sync.dma_start(out=dstd[:, 0:2048],
                                              in_=st[p0:p0 + 64, 0:2048])
                            nc.scalar.dma_start(out=dstd[:, 2048:4096],
                                                in_=st[p0:p0 + 64, 2048:4096])
    nc.compile()
    return nc


_cache = {}


def kernel(**inputs):
    x = np.ascontiguousarray(np.asarray(inputs["x"], np.float32))
    params = {k: np.asarray(v, np.float32) for k, v in inputs.items()
              if k != "x"}

    key = hash(tuple(sorted((k, v.tobytes()) for k, v in params.items())))
    if key not in _cache:
        _cache[key] = build(params)
    nc = _cache[key]

    in_maps = [{"x": np.ascontiguousarray(x[B * i:B * (i + 1)])}
               for i in range(NCORES)]
    res = run_bass_kernel_spmd(nc, in_maps, core_ids=list(range(NCORES)),
                               trace=KERNEL_TRACE)
    out = np.concatenate([res.results[i]["out"] for i in range(NCORES)], axis=0)
    if KERNEL_TRACE:
        kernel.last_result = res
    return out
